# revision 1
# baseline (speedup 1.0000x reference)
"""ClassCapsule dynamic-routing kernel for 8 Trainium2 NeuronCores.

Problem (hardcoded shapes):
    x:    [64, 2048, 16]  fp32
    W:    [2048, 16, 1024] fp32
    bias: [64, 16]        fp32
    out:  [64, 64, 16]    fp32  (squeezed v after 3 routing iterations)

Strategy (batch-sharded, no collectives):
  - B=64 split across 8 cores (8 batches each).
  - u_hat = einsum('bij,ijk->bik') computed on the PE via a block-diagonal
    trick: 8 in_caps share one matmul; lhsT is a host-prepared block-diagonal
    arrangement of x with K=(i_sub,e)=128, M=(i_sub,b)=64.
  - u_hat tiles [128=(i_sub16,b8), 1024] stored to DRAM (bf16), re-read for
    the 2 remaining routing iterations.
  - Routing per tile: agreement = reduce_d(u_hat*v) (vector), softmax over
    n_caps (ACT exp + vector reciprocal), weighted sum over in_caps via a
    constant 0/1 selector matmul on the PE accumulating in PSUM.
"""

import numpy as np

import concourse.bass as bass
import concourse.tile as tile
from concourse import bacc, mybir
from concourse.bass_utils import run_bass_kernel_spmd

# ---------------------------------------------------------------- constants
B, IC, E = 64, 2048, 16          # batch, in_caps, in_dim
NCAP, D = 64, 16                 # n_caps, cap_dim
ND = NCAP * D                    # 1024
CORES = 8
BL = B // CORES                  # 8 local batches
IB8 = IC // 8                    # 256 blocks of 8 in_caps (matmul granularity)
NT = IC // 16                    # 128 u_hat tiles of 16 in_caps
EPS = 1e-7

FP = mybir.dt.float32
BF = mybir.dt.bfloat16


def _host_prep(x, W, bias):
    """Build per-core host-side tensors."""
    # Block-diagonal x for the projection matmuls.
    # lhsT[blk][(i_sub*16+e), (j_sub*8+b)] = x[b, blk*8+j_sub, e] * (i_sub==j_sub)
    # -> per core: [IB8, 128, 64] fp32
    w_r = W.reshape(IB8, 8 * E, ND)  # [256, 128, 1024]
    # wx[blk] = [128, 1024 + 64]: W block columns then block-diagonal x columns,
    # so ONE dma per block feeds both matmul operands (single sync wait on PE).
    wx_all = []
    for c in range(CORES):
        xc = x[c * BL:(c + 1) * BL]                      # [8, 2048, 16]
        wx = np.zeros((IB8, 128, ND + 8 * BL), dtype=np.float32)
        wx[:, :, :ND] = w_r
        # fill diagonal blocks: rows i_sub*16+e, cols ND + i_sub*8+b
        xr = xc.transpose(1, 2, 0).reshape(IB8, 8, E, BL)  # [blk, i_sub, e, b]
        for s in range(8):
            wx[:, s * E:(s + 1) * E, ND + s * BL:ND + (s + 1) * BL] = xr[:, s]
        wx_all.append(wx)

    # selector: sel8[p, b] = 1 if p % 8 == b   (partition p = i_sub*8 + b)
    sel8 = np.zeros((128, BL), dtype=np.float32)
    sel8[np.arange(128), np.arange(128) % BL] = 1.0

    bias_f = np.tile(bias.reshape(1, ND), (BL, 1)).astype(np.float32)  # [8, 1024]
    return wx_all, sel8, bias_f


def _build_program():
    nc = bacc.Bacc("TRN2", target_bir_lowering=False)

    wx_d = nc.dram_tensor("wx", [IB8, 128, ND + 8 * BL], FP, kind="ExternalInput")
    sel8_d = nc.dram_tensor("sel8", [128, BL], FP, kind="ExternalInput")
    bias_d = nc.dram_tensor("bias_f", [BL, ND], FP, kind="ExternalInput")
    v_out = nc.dram_tensor("v_out", [BL, ND], FP, kind="ExternalOutput")

    u_hat_d = nc.dram_tensor("u_hat_d", [NT, 128, ND], BF)   # internal scratch
    v_scr = nc.dram_tensor("v_scr", [BL, ND], BF)            # bcast bounce

    with tile.TileContext(nc) as tc:
        with (
            tc.tile_pool(name="wp", bufs=4) as wp,
            tc.tile_pool(name="up", bufs=3) as up,
            tc.tile_pool(name="tp", bufs=3) as tp,
            tc.tile_pool(name="smalls", bufs=4) as sp,
            tc.tile_pool(name="consts", bufs=1) as cp,
            tc.tile_pool(name="vb", bufs=2) as vbp,
            tc.tile_pool(name="ps", bufs=2, space="PSUM") as psp,
            tc.tile_pool(name="ps_acc", bufs=1, space="PSUM") as psa,
            tc.tile_pool(name="bstate", bufs=1) as bsp,
        ):
            # ---- constants resident in SBUF
            sel8_f = cp.tile([128, BL], FP)
            nc.sync.dma_start(out=sel8_f, in_=sel8_d[:, :])
            sel8_b = cp.tile([128, BL], BF)
            nc.scalar.copy(out=sel8_b, in_=sel8_f)
            bias_sb = cp.tile([BL, ND], FP)
            nc.sync.dma_start(out=bias_sb, in_=bias_d[:, :])
            eps_t = cp.tile([BL, 1], FP)
            nc.vector.memset(eps_t, EPS)

            # routing logits state: [128, NT*64]
            b_all = bsp.tile([128, NT * NCAP], FP)

            # ---------------- squash helper: v = squash(s_psum*scale + bias)
            def squash_from_psum(s_ps, scale):
                s_sb = sp.tile([BL, ND], FP, tag="s_sb")
                # s = s_ps*scale + bias
                nc.vector.scalar_tensor_tensor(
                    out=s_sb, in0=s_ps, scalar=float(scale), in1=bias_sb,
                    op0=mybir.AluOpType.mult, op1=mybir.AluOpType.add)
                sq = sp.tile([BL, ND], FP, tag="sq")
                nc.vector.tensor_mul(sq, s_sb, s_sb)
                nsq = sp.tile([BL, NCAP], FP, tag="nsq")
                nc.vector.reduce_sum(
                    out=nsq, in_=sq.rearrange("p (n d) -> p n d", d=D),
                    axis=mybir.AxisListType.X)
                norm = sp.tile([BL, NCAP], FP, tag="norm")
                # norm = sqrt(nsq + EPS)
                nc.scalar.activation(out=norm, in_=nsq,
                                     func=mybir.ActivationFunctionType.Sqrt,
                                     bias=eps_t[:, :], scale=1.0)
                den = sp.tile([BL, NCAP], FP, tag="den")
                # den = (nsq + EPS + 1) * norm
                nc.vector.scalar_tensor_tensor(
                    out=den, in0=nsq, scalar=float(EPS + 1.0), in1=norm,
                    op0=mybir.AluOpType.add, op1=mybir.AluOpType.mult)
                rden = sp.tile([BL, NCAP], FP, tag="rden")
                nc.vector.reciprocal(out=rden, in_=den)
                fac = sp.tile([BL, NCAP], FP, tag="fac")
                # fac = (nsq + EPS) * rden
                nc.vector.scalar_tensor_tensor(
                    out=fac, in0=nsq, scalar=float(EPS), in1=rden,
                    op0=mybir.AluOpType.add, op1=mybir.AluOpType.mult)
                v_sb = sp.tile([BL, ND], FP, tag="v_sb")
                fac_b = bass.AP(tensor=fac.tensor, offset=fac.offset,
                                ap=[list(fac.ap[0]), list(fac.ap[1]), [0, D]])
                nc.vector.tensor_mul(
                    v_sb.rearrange("p (n d) -> p n d", d=D),
                    s_sb.rearrange("p (n d) -> p n d", d=D),
                    fac_b)
                return s_sb, v_sb

            def broadcast_v(v_sb):
                """v_sb [8, 1024] fp32 -> vb [128, 1024] bf16 (partition bcast)."""
                v_bf = sp.tile([BL, ND], BF, tag="v_bf")
                nc.vector.tensor_copy(out=v_bf, in_=v_sb)
                nc.sync.dma_start(out=v_scr[:, :], in_=v_bf)
                vb = vbp.tile([128, ND], BF, tag="vb")
                src = bass.AP(tensor=v_scr, offset=0,
                              ap=[[0, 128 // BL], [ND, BL], [1, ND]])
                nc.sync.dma_start(out=vb, in_=src)
                return vb

            # ================= Phase P: projection + iter-0 sum =================
            s0_ps = psa.tile([BL, ND], FP, tag="s_acc")
            for t in range(NT):
                u_ps = psp.tile([128, ND], FP, tag="u_ps")
                for h in range(2):  # two 8-in_cap blocks -> partitions h*64..
                    blk = 2 * t + h
                    wt = wp.tile([128, ND + 8 * BL], FP, tag="w")
                    nc.sync.dma_start(out=wt, in_=wx_d[blk])
                    for nh in range(2):  # N halves of 512
                        nc.tensor.matmul(
                            u_ps[h * 64:(h + 1) * 64, nh * 512:(nh + 1) * 512],
                            wt[:, ND:ND + 8 * BL],
                            wt[:, nh * 512:(nh + 1) * 512],
                            start=True, stop=True)
                u_bf = up.tile([128, ND], BF, tag="u_bf")
                nc.scalar.copy(out=u_bf[:, 0:512], in_=u_ps[:, 0:512])
                nc.scalar.copy(out=u_bf[:, 512:1024], in_=u_ps[:, 512:1024])
                nc.sync.dma_start(out=u_hat_d[t], in_=u_bf)
                for nh in range(2):
                    nc.tensor.matmul(
                        s0_ps[:, nh * 512:(nh + 1) * 512],
                        sel8_b, u_bf[:, nh * 512:(nh + 1) * 512],
                        start=(t == 0), stop=(t == NT - 1),
                        skip_group_check=True)

            _, v_sb = squash_from_psum(s0_ps, 1.0 / NCAP)
            vb = broadcast_v(v_sb)

            # ================= Routing iterations 1 and 2 =================
            for it in (1, 2):
                s_ps = psa.tile([BL, ND], FP, tag="s_acc")
                for t in range(NT):
                    u_bf = up.tile([128, ND], BF, tag="u_bf")
                    nc.sync.dma_start(out=u_bf, in_=u_hat_d[t])
                    tmp = tp.tile([128, ND], BF, tag="tmp")
                    nc.gpsimd.tensor_mul(tmp, u_bf, vb)
                    b_slice = b_all[:, t * NCAP:(t + 1) * NCAP]
                    if it == 1:
                        # b starts at zero: agreement goes straight into b
                        nc.vector.reduce_sum(
                            out=b_slice,
                            in_=tmp.rearrange("p (n d) -> p n d", d=D),
                            axis=mybir.AxisListType.X)
                    else:
                        agr = sp.tile([128, NCAP], FP, tag="agr")
                        nc.vector.reduce_sum(
                            out=agr,
                            in_=tmp.rearrange("p (n d) -> p n d", d=D),
                            axis=mybir.AxisListType.X)
                        nc.vector.tensor_add(b_slice, b_slice, agr)
                    c_un = sp.tile([128, NCAP], FP, tag="c_un")
                    se = sp.tile([128, 1], FP, tag="se")
                    nc.scalar.activation(out=c_un, in_=b_slice,
                                         func=mybir.ActivationFunctionType.Exp,
                                         accum_out=se)
                    rec = sp.tile([128, 1], FP, tag="rec")
                    nc.vector.reciprocal(out=rec, in_=se)
                    c_bf = sp.tile([128, NCAP], BF, tag="c_bf")
                    nc.scalar.mul(c_bf, c_un, rec)
                    w_bf = tp.tile([128, ND], BF, tag="w_bf")
                    c_b = bass.AP(tensor=c_bf.tensor, offset=c_bf.offset,
                                  ap=[list(c_bf.ap[0]), list(c_bf.ap[1]), [0, D]])
                    nc.vector.tensor_mul(
                        w_bf.rearrange("p (n d) -> p n d", d=D),
                        u_bf.rearrange("p (n d) -> p n d", d=D),
                        c_b)
                    for nh in range(2):
                        nc.tensor.matmul(
                            s_ps[:, nh * 512:(nh + 1) * 512],
                            sel8_b, w_bf[:, nh * 512:(nh + 1) * 512],
                            start=(t == 0), stop=(t == NT - 1),
                            skip_group_check=True)
                _, v_sb = squash_from_psum(s_ps, 1.0)
                if it < 2:
                    vb = broadcast_v(v_sb)
                else:
                    nc.sync.dma_start(out=v_out[:, :], in_=v_sb)

    nc.compile()
    return nc


_CACHED = {}


def _get_program():
    if "nc" not in _CACHED:
        _CACHED["nc"] = _build_program()
    return _CACHED["nc"]


def kernel(x, W, bias):
    x = np.asarray(x, dtype=np.float32)
    W = np.asarray(W, dtype=np.float32)
    bias = np.asarray(bias, dtype=np.float32)

    wx_all, sel8, bias_f = _host_prep(x, W, bias)
    nc = _get_program()

    in_maps = []
    for c in range(CORES):
        in_maps.append({
            "wx": wx_all[c],
            "sel8": sel8,
            "bias_f": bias_f,
        })
    res = run_bass_kernel_spmd(nc, in_maps, core_ids=list(range(CORES)))
    _CACHED["last_results"] = res
    outs = [res.results[c]["v_out"].reshape(BL, NCAP, D) for c in range(CORES)]
    return np.concatenate(outs, axis=0)



# revision 4
# speedup vs baseline: 1.5556x; 1.5556x over previous
"""ClassCapsule dynamic-routing kernel for 8 Trainium2 NeuronCores.

Problem (hardcoded shapes):
    x:    [64, 2048, 16]  fp32
    W:    [2048, 16, 1024] fp32
    bias: [64, 16]        fp32
    out:  [64, 64, 16]    fp32  (squeezed v after 3 routing iterations)

Strategy (in_caps-sharded, s-AllReduce per iteration):
  - IC=2048 in_caps split across 8 cores (256 each); every core holds all
    64 batches.  W traffic per core is 1/8th of the replicated layout.
  - All inputs shipped bf16: per-core wxbd[blk] packs W for an 8-in_cap
    block ([128,1024]) plus the block-diagonal x operand ([128,512],
    4 groups of 16 batches) -> one DMA feeds both matmul operands.
  - u_hat = x @ W on the PE via the block-diagonal trick: K=(s,e)=128,
    M=(s,b16)=128 per matmul, all bf16.  Iteration-0 s is accumulated
    during projection with a 0/1 selector matmul (c0 is uniform).
  - u_hat (bf16) kept SBUF-resident for the first RES tiles, spilled to
    DRAM for the rest and re-read in iterations 1-2.
  - Per iteration: agreement = reduce_d(u*v) (DVE), softmax over n_caps
    (ACT exp + DVE reciprocal), cu = u*c (DVE), s = selector matmul (PE),
    then an 8-core AllReduce of the partial s [64,1024] fp32, squash on
    the allreduced s (identical on every core).
"""

import numpy as np
import ml_dtypes

import concourse.bass as bass
import concourse.tile as tile
from concourse import bacc, mybir
from concourse.bass_utils import run_bass_kernel_spmd

# ---------------------------------------------------------------- constants
B, IC, E = 64, 2048, 16          # batch, in_caps, in_dim
NCAP, D = 64, 16                 # n_caps, cap_dim
ND = NCAP * D                    # 1024
CORES = 8
ICL = IC // CORES                # 256 local in_caps
NBLK = ICL // 8                  # 32 blocks of 8 in_caps
NT = NBLK * 4                    # 128 u_hat tiles [(s,b16), 1024]
RES = 48                         # tiles resident in SBUF; rest spilled
EPS = 1e-7

FP = mybir.dt.float32
BF = mybir.dt.bfloat16
BF_NP = ml_dtypes.bfloat16


def _host_prep(x, W, bias):
    """Build per-core host-side tensors (all bf16 except bias)."""
    wxbd_all = []
    for c in range(CORES):
        i0 = c * ICL
        w8 = W[i0:i0 + ICL].reshape(NBLK, 128, ND)          # [(blk),(s,e),nd]
        xc = x[:, i0:i0 + ICL, :]                           # [64, 256, 16]
        # xr[blk, s, e, bg, b] = x[bg*16+b, i0+blk*8+s, e]
        xr = xc.transpose(1, 2, 0).reshape(NBLK, 8, E, 4, 16)
        xbd = np.zeros((NBLK, 128, 4, 128), np.float32)
        for s in range(8):
            xbd[:, s * E:(s + 1) * E, :, s * 16:(s + 1) * 16] = xr[:, s]
        wxbd = np.concatenate(
            [w8, xbd.reshape(NBLK, 128, 512)], axis=2)      # [32, 128, 1536]
        wxbd_all.append(wxbd.astype(BF_NP))

    # selector: sel[p, g*64 + m] = 1 if m == g*16 + p%16
    sel = np.zeros((128, 4 * NCAP), np.float32)
    p = np.arange(128)
    for g in range(4):
        sel[p, g * NCAP + g * 16 + (p % 16)] = 1.0
    sel = sel.astype(BF_NP)

    bias_flat = np.ascontiguousarray(bias.reshape(1, ND)).astype(np.float32)
    return wxbd_all, sel, bias_flat


def _build_program():
    nc = bacc.Bacc("TRN2", target_bir_lowering=False, num_devices=CORES)

    wxbd_d = nc.dram_tensor("wxbd", [NBLK, 128, ND + 512], BF,
                            kind="ExternalInput")
    sel_d = nc.dram_tensor("sel", [128, 4 * NCAP], BF, kind="ExternalInput")
    bias_d = nc.dram_tensor("bias_flat", [1, ND], FP, kind="ExternalInput")
    v_out = nc.dram_tensor("v_out", [B, ND], FP, kind="ExternalOutput")

    u_spill_d = nc.dram_tensor("u_spill", [NT - RES, 128, ND], BF)
    v_scr = [nc.dram_tensor(f"v_scr{r}", [B, ND], BF) for r in range(2)]
    ar_in = [nc.dram_tensor(f"ar_in{r}", [B, ND], FP) for r in range(3)]
    ar_out = [nc.dram_tensor(f"ar_out{r}", [B, ND], FP, addr_space="Shared")
              for r in range(3)]

    with tile.TileContext(nc) as tc:
        with (
            tc.tile_pool(name="wp", bufs=3) as wp,
            tc.tile_pool(name="ul", bufs=3) as ulp,        # spill load/stage
            tc.tile_pool(name="tp", bufs=3) as tp,         # u*v products
            tc.tile_pool(name="cup", bufs=3) as cup,       # u*c products
            tc.tile_pool(name="smalls", bufs=4) as sp,
            tc.tile_pool(name="sq", bufs=1) as qp,         # squash [64,*]
            tc.tile_pool(name="consts", bufs=1) as cp,
            tc.tile_pool(name="vb", bufs=1) as vbp,
            tc.tile_pool(name="ures", bufs=1) as urp,
            tc.tile_pool(name="bstate", bufs=1) as bsp,
            tc.tile_pool(name="ps", bufs=2, space="PSUM") as psp,
            tc.tile_pool(name="ps_acc", bufs=1, space="PSUM") as psa,
        ):
            # ---- constants resident in SBUF
            sel_sb = cp.tile([128, 4 * NCAP], BF)
            nc.sync.dma_start(out=sel_sb, in_=sel_d[:, :])
            bias_sb = cp.tile([B, ND], FP)
            bias_src = bass.AP(tensor=bias_d, offset=0, ap=[[0, B], [1, ND]])
            nc.sync.dma_start(out=bias_sb, in_=bias_src)
            eps_t = cp.tile([B, 1], FP)
            nc.vector.memset(eps_t, EPS)

            # persistent state
            u_res = urp.tile([128, RES * ND], BF)          # resident u_hat
            b_all = bsp.tile([128, NT * NCAP], FP)         # routing logits

            def u_slice(t, stage=None):
                if t < RES:
                    return u_res[:, t * ND:(t + 1) * ND]
                return stage

            # ---------------- squash helper: v = squash(s*scale + bias)
            def squash(s_in, scale):
                s_sb = qp.tile([B, ND], FP, tag="s_sb")
                nc.vector.scalar_tensor_tensor(
                    out=s_sb, in0=s_in, scalar=float(scale), in1=bias_sb,
                    op0=mybir.AluOpType.mult, op1=mybir.AluOpType.add)
                sq = qp.tile([B, ND], FP, tag="sq")
                nc.vector.tensor_mul(sq, s_sb, s_sb)
                nsq = sp.tile([B, NCAP], FP, tag="nsq")
                nc.vector.reduce_sum(
                    out=nsq, in_=sq.rearrange("p (n d) -> p n d", d=D),
                    axis=mybir.AxisListType.X)
                norm = sp.tile([B, NCAP], FP, tag="norm")
                nc.scalar.activation(out=norm, in_=nsq,
                                     func=mybir.ActivationFunctionType.Sqrt,
                                     bias=eps_t[:, :], scale=1.0)
                den = sp.tile([B, NCAP], FP, tag="den")
                nc.vector.scalar_tensor_tensor(
                    out=den, in0=nsq, scalar=float(EPS + 1.0), in1=norm,
                    op0=mybir.AluOpType.add, op1=mybir.AluOpType.mult)
                rden = sp.tile([B, NCAP], FP, tag="rden")
                nc.vector.reciprocal(out=rden, in_=den)
                fac = sp.tile([B, NCAP], FP, tag="fac")
                nc.vector.scalar_tensor_tensor(
                    out=fac, in0=nsq, scalar=float(EPS), in1=rden,
                    op0=mybir.AluOpType.add, op1=mybir.AluOpType.mult)
                v_sb = qp.tile([B, ND], FP, tag="v_sb")
                fac_b = bass.AP(tensor=fac.tensor, offset=fac.offset,
                                ap=[list(fac.ap[0]), list(fac.ap[1]), [0, D]])
                nc.vector.tensor_mul(
                    v_sb.rearrange("p (n d) -> p n d", d=D),
                    s_sb.rearrange("p (n d) -> p n d", d=D),
                    fac_b)
                return v_sb

            def allreduce_s(s_ps, r):
                """partial s (PSUM or SBUF) -> allreduced SBUF fp32 tile."""
                s_loc = qp.tile([B, ND], FP, tag="s_loc")
                nc.vector.tensor_copy(out=s_loc, in_=s_ps)
                nc.sync.dma_start(out=ar_in[r][:, :], in_=s_loc)
                nc.gpsimd.collective_compute(
                    "AllReduce",
                    mybir.AluOpType.add,
                    replica_groups=[list(range(CORES))],
                    ins=[ar_in[r][:, :].opt()],
                    outs=[ar_out[r][:, :].opt()],
                )
                s_glob = qp.tile([B, ND], FP, tag="s_glob")
                nc.sync.dma_start(out=s_glob, in_=ar_out[r][:, :])
                return s_glob

            def broadcast_v(v_sb, r):
                """v [64,1024] fp32 -> vb [128, 4*1024] bf16 (per-bg bcast)."""
                v_bf = qp.tile([B, ND], BF, tag="v_bf")
                nc.vector.tensor_copy(out=v_bf, in_=v_sb)
                nc.sync.dma_start(out=v_scr[r][:, :], in_=v_bf)
                vb = vbp.tile([128, 4 * ND], BF, tag="vb")
                for g in range(4):
                    src = bass.AP(tensor=v_scr[r], offset=g * 16 * ND,
                                  ap=[[0, 8], [ND, 16], [1, ND]])
                    nc.sync.dma_start(out=vb[:, g * ND:(g + 1) * ND], in_=src)
                return vb

            # ================= Phase P: projection + iter-0 sum =============
            s0_ps = psa.tile([B, ND], FP, tag="s_acc")
            for blk in range(NBLK):
                wt = wp.tile([128, ND + 512], BF, tag="w")
                nc.sync.dma_start(out=wt, in_=wxbd_d[blk])
                for g in range(4):
                    t = blk * 4 + g
                    u_ps = psp.tile([128, ND], FP, tag="u_ps")
                    for nh in range(2):
                        nc.tensor.matmul(
                            u_ps[:, nh * 512:(nh + 1) * 512],
                            wt[:, ND + g * 128:ND + (g + 1) * 128],
                            wt[:, nh * 512:(nh + 1) * 512],
                            start=True, stop=True)
                    if t < RES:
                        u_bf = u_res[:, t * ND:(t + 1) * ND]
                    else:
                        u_bf = ulp.tile([128, ND], BF, tag="u_ld", name="u_bf")
                    nc.scalar.copy(out=u_bf[:, 0:512], in_=u_ps[:, 0:512])
                    nc.vector.tensor_copy(out=u_bf[:, 512:1024],
                                          in_=u_ps[:, 512:1024])
                    if t >= RES:
                        nc.sync.dma_start(out=u_spill_d[t - RES], in_=u_bf)
                    for nh in range(2):
                        nc.tensor.matmul(
                            s0_ps[:, nh * 512:(nh + 1) * 512],
                            sel_sb[:, g * NCAP:(g + 1) * NCAP],
                            u_bf[:, nh * 512:(nh + 1) * 512],
                            start=(t == 0), stop=(t == NT - 1),
                            skip_group_check=True)

            s_glob = allreduce_s(s0_ps, 0)
            v_sb = squash(s_glob, 1.0 / NCAP)
            vb = broadcast_v(v_sb, 0)

            # ================= Routing iterations 1 and 2 ===================
            for it in (1, 2):
                s_ps = psa.tile([B, ND], FP, tag="s_acc")
                for t in range(NT):
                    g = t % 4
                    if t < RES:
                        u_bf = u_res[:, t * ND:(t + 1) * ND]
                    else:
                        u_bf = ulp.tile([128, ND], BF, tag="u_ld")
                        nc.sync.dma_start(out=u_bf, in_=u_spill_d[t - RES])
                    tmp = tp.tile([128, ND], BF, tag="tmp")
                    nc.vector.tensor_mul(tmp, u_bf, vb[:, g * ND:(g + 1) * ND])
                    b_slice = b_all[:, t * NCAP:(t + 1) * NCAP]
                    if it == 1:
                        nc.vector.reduce_sum(
                            out=b_slice,
                            in_=tmp.rearrange("p (n d) -> p n d", d=D),
                            axis=mybir.AxisListType.X)
                    else:
                        agr = sp.tile([128, NCAP], FP, tag="agr")
                        nc.vector.reduce_sum(
                            out=agr,
                            in_=tmp.rearrange("p (n d) -> p n d", d=D),
                            axis=mybir.AxisListType.X)
                        nc.vector.tensor_add(b_slice, b_slice, agr)
                    c_un = sp.tile([128, NCAP], FP, tag="c_un")
                    se = sp.tile([128, 1], FP, tag="se")
                    nc.scalar.activation(out=c_un, in_=b_slice,
                                         func=mybir.ActivationFunctionType.Exp,
                                         accum_out=se)
                    rec = sp.tile([128, 1], FP, tag="rec")
                    nc.vector.reciprocal(out=rec, in_=se)
                    c_bf = sp.tile([128, NCAP], BF, tag="c_bf")
                    nc.scalar.mul(c_bf, c_un, rec)
                    cu = cup.tile([128, ND], BF, tag="cu")
                    c_b = bass.AP(tensor=c_bf.tensor, offset=c_bf.offset,
                                  ap=[list(c_bf.ap[0]), list(c_bf.ap[1]),
                                      [0, D]])
                    nc.vector.tensor_mul(
                        cu.rearrange("p (n d) -> p n d", d=D),
                        u_bf.rearrange("p (n d) -> p n d", d=D),
                        c_b)
                    for nh in range(2):
                        nc.tensor.matmul(
                            s_ps[:, nh * 512:(nh + 1) * 512],
                            sel_sb[:, g * NCAP:(g + 1) * NCAP],
                            cu[:, nh * 512:(nh + 1) * 512],
                            start=(t == 0), stop=(t == NT - 1),
                            skip_group_check=True)
                s_glob = allreduce_s(s_ps, it)
                v_sb = squash(s_glob, 1.0)
                if it < 2:
                    vb = broadcast_v(v_sb, 1)
                else:
                    nc.sync.dma_start(out=v_out[:, :], in_=v_sb)

    nc.compile()
    return nc


_CACHED = {}


def _get_program():
    if "nc" not in _CACHED:
        _CACHED["nc"] = _build_program()
    return _CACHED["nc"]


def kernel(x, W, bias):
    x = np.asarray(x, dtype=np.float32)
    W = np.asarray(W, dtype=np.float32)
    bias = np.asarray(bias, dtype=np.float32)

    wxbd_all, sel, bias_flat = _host_prep(x, W, bias)
    nc = _get_program()

    in_maps = []
    for c in range(CORES):
        in_maps.append({
            "wxbd": wxbd_all[c],
            "sel": sel,
            "bias_flat": bias_flat,
        })
    res = run_bass_kernel_spmd(nc, in_maps, core_ids=list(range(CORES)))
    _CACHED["last_results"] = res
    return res.results[0]["v_out"].reshape(B, NCAP, D).astype(np.float32)


# revision 18
# speedup vs baseline: 2.0727x; 1.3324x over previous
"""ClassCapsule dynamic-routing kernel for 8 Trainium2 NeuronCores.

Problem (hardcoded shapes):
    x:    [64, 2048, 16]  fp32
    W:    [2048, 16, 1024] fp32
    bias: [64, 16]        fp32
    out:  [64, 64, 16]    fp32  (squeezed v after 3 routing iterations)

Strategy (in_caps-sharded, s-AllReduce per iteration):
  - IC=2048 in_caps split across 8 cores (256 each); every core holds all
    64 batches.  W traffic per core is 1/8th of the replicated layout.
  - All inputs shipped bf16: per-core wxbd[blk] packs W for an 8-in_cap
    block ([128,1024]), the block-diagonal x operand ([128,512], 4 groups
    of 16 batches) and the dense x operand ([128,64]) -> one DMA feeds
    all matmul operands for a block.
  - u_hat = x @ W on the PE via the block-diagonal trick: K=(s,e)=128,
    M=(s,b16)=128 per matmul, all bf16.  Iteration-0 s (c0 uniform) is a
    plain dense GEMM over K=(in_cap,e): s0 += xk[blk].T @ W8[blk], which
    keeps the PE warm and off the copy critical path.
  - u_hat (bf16) kept SBUF-resident for the first RESB block groups,
    spilled to DRAM per block group [128,4096] and re-read in iters 1-2.
  - Per iteration, per block group: agreement products (GPSIMD for 3 of
    4 groups, DVE else), d-reduction via an in-place pairwise bf16 add
    tree on DVE (2x mode), batched softmax (ACT exp, DVE smalls), c
    replicated over cap_dim on ACT, cu product on DVE (2x), s
    accumulated with per-tile selector matmuls (PE), then an 8-core
    fp32 AllReduce of the partial s [64,1024], squash on every core.
"""

import numpy as np
import ml_dtypes

import concourse.bass as bass
import concourse.tile as tile
from concourse import bacc, mybir
from concourse.bass_utils import run_bass_kernel_spmd

# ---------------------------------------------------------------- constants
B, IC, E = 64, 2048, 16          # batch, in_caps, in_dim
NCAP, D = 64, 16                 # n_caps, cap_dim
ND = NCAP * D                    # 1024
CORES = 8
ICL = IC // CORES                # 256 local in_caps
NBLK = ICL // 8                  # 32 blocks of 8 in_caps
NT = NBLK * 4                    # 128 u_hat tiles [(s,b16), 1024]
RESB = 7                         # block groups resident in SBUF
GND = 4 * ND                     # group width (4096)
WCOL = ND + B                    # wxbd columns (1088)
EPS = 1e-7

FP = mybir.dt.float32
BF = mybir.dt.bfloat16
BF_NP = ml_dtypes.bfloat16


def _host_prep(x, W, bias):
    """Build per-core host-side tensors (all bf16 except bias)."""
    wxbd_all = []
    x2bd_all = []
    for c in range(CORES):
        i0 = c * ICL
        w8 = W[i0:i0 + ICL].reshape(NBLK, 128, ND)          # [(blk),(s,e),nd]
        xc = x[:, i0:i0 + ICL, :]                           # [64, 256, 16]
        # xr[blk, s, e, bg, b] = x[bg*16+b, i0+blk*8+s, e]
        xr = xc.transpose(1, 2, 0).reshape(NBLK, 8, E, 4, 16)
        xk = xr.reshape(NBLK, 128, B)                       # [(s,e), b]
        wxbd = np.concatenate([w8, xk], axis=2)             # [32, 128, 1088]
        wxbd_all.append(wxbd.astype(BF_NP))
        # x2bd[32r+16j'+e, (blk*4+bg)*32 + 16j + b] =
        #     x[bg*16+b, i0+blk*8+2r+j, e] * (j == j')
        xr2 = xr.reshape(NBLK, 4, 2, E, 4, 16)              # [blk,r,j,e,bg,b]
        x2 = np.zeros((4, 2, E, NBLK, 4, 2, 16), np.float32)
        for j in range(2):
            x2[:, j, :, :, :, j, :] = xr2[:, :, j].transpose(1, 2, 0, 3, 4)
        x2bd_all.append(x2.reshape(128, NBLK * 128).astype(BF_NP))

    # selector: sel[p, g*64 + m] = 1 if m == g*16 + p%16
    sel = np.zeros((128, 4 * NCAP), np.float32)
    p = np.arange(128)
    for g in range(4):
        sel[p, g * NCAP + g * 16 + (p % 16)] = 1.0
    sel = sel.astype(BF_NP)

    bias_flat = np.ascontiguousarray(bias.reshape(1, ND)).astype(np.float32)
    return wxbd_all, x2bd_all, sel, bias_flat


def _build_program():
    nc = bacc.Bacc("TRN2", target_bir_lowering=False, num_devices=CORES)

    wxbd_d = nc.dram_tensor("wxbd", [NBLK, 128, WCOL], BF,
                            kind="ExternalInput")
    x2bd_d = nc.dram_tensor("x2bd", [128, NBLK * 128], BF,
                            kind="ExternalInput")
    sel_d = nc.dram_tensor("sel", [128, 4 * NCAP], BF, kind="ExternalInput")
    bias_d = nc.dram_tensor("bias_flat", [1, ND], FP, kind="ExternalInput")
    v_out = nc.dram_tensor("v_out", [B, ND], FP, kind="ExternalOutput")

    u_spill_d = nc.dram_tensor("u_spill", [NBLK - RESB, 128, GND], BF)
    v_scr = [nc.dram_tensor(f"v_scr{r}", [B, ND], BF) for r in range(2)]
    ar_in = [nc.dram_tensor(f"ar_in{r}", [B, ND], FP) for r in range(3)]
    ar_out = [nc.dram_tensor(f"ar_out{r}", [B, ND], FP, addr_space="Shared")
              for r in range(3)]

    with tile.TileContext(nc) as tc:
        with (
            tc.tile_pool(name="wp", bufs=2) as wp,
            tc.tile_pool(name="ul", bufs=4) as ulp,        # spill load/stage
            tc.tile_pool(name="tp", bufs=2) as tp,         # u*v products
            tc.tile_pool(name="crp", bufs=2) as crp,       # c replicated
            tc.tile_pool(name="cup", bufs=2) as cup,       # u*c products
            tc.tile_pool(name="smalls", bufs=2) as sp,
            tc.tile_pool(name="sq", bufs=1) as qp,         # squash [64,*]
            tc.tile_pool(name="consts", bufs=1) as cp,
            tc.tile_pool(name="vb", bufs=1) as vbp,
            tc.tile_pool(name="ures", bufs=1) as urp,
            tc.tile_pool(name="bstate", bufs=1) as bsp,
            tc.tile_pool(name="ps", bufs=3, space="PSUM") as psp,
            tc.tile_pool(name="ps_acc", bufs=1, space="PSUM") as psa,
        ):
            # ---- constants resident in SBUF
            sel_sb = cp.tile([128, 4 * NCAP], BF)
            nc.sync.dma_start(out=sel_sb, in_=sel_d[:, :])
            x2bd = cp.tile([128, NBLK * 128], BF)
            nc.sync.dma_start(out=x2bd, in_=x2bd_d[:, :])
            bias_sb = cp.tile([B, ND], FP)
            bias_src = bass.AP(tensor=bias_d, offset=0, ap=[[0, B], [1, ND]])
            nc.sync.dma_start(out=bias_sb, in_=bias_src)
            eps_t = cp.tile([B, 1], FP)
            nc.vector.memset(eps_t, EPS)

            # persistent state
            u_res = urp.tile([128, RESB * GND], BF)        # resident u_hat
            b_all = bsp.tile([128, NT * NCAP], BF)         # routing logits

            # ---------------- squash helper: v = squash(s*scale + bias)
            def squash(s_in, scale):
                s_sb = qp.tile([B, ND], FP, tag="s_sb")
                nc.vector.scalar_tensor_tensor(
                    out=s_sb, in0=s_in, scalar=float(scale), in1=bias_sb,
                    op0=mybir.AluOpType.mult, op1=mybir.AluOpType.add)
                sq = qp.tile([B, ND], FP, tag="sq")
                nc.vector.tensor_mul(sq, s_sb, s_sb)
                nsq = sp.tile([B, NCAP], FP, tag="nsq")
                nc.vector.reduce_sum(
                    out=nsq, in_=sq.rearrange("p (n d) -> p n d", d=D),
                    axis=mybir.AxisListType.X)
                norm = sp.tile([B, NCAP], FP, tag="norm")
                nc.scalar.activation(out=norm, in_=nsq,
                                     func=mybir.ActivationFunctionType.Sqrt,
                                     bias=eps_t[:, :], scale=1.0)
                den = sp.tile([B, NCAP], FP, tag="den")
                nc.vector.scalar_tensor_tensor(
                    out=den, in0=nsq, scalar=float(EPS + 1.0), in1=norm,
                    op0=mybir.AluOpType.add, op1=mybir.AluOpType.mult)
                rden = sp.tile([B, NCAP], FP, tag="rden")
                nc.vector.reciprocal(out=rden, in_=den)
                fac = sp.tile([B, NCAP], FP, tag="fac")
                nc.vector.scalar_tensor_tensor(
                    out=fac, in0=nsq, scalar=float(EPS), in1=rden,
                    op0=mybir.AluOpType.add, op1=mybir.AluOpType.mult)
                v_sb = qp.tile([B, ND], FP, tag="v_sb")
                fac_b = bass.AP(tensor=fac.tensor, offset=fac.offset,
                                ap=[list(fac.ap[0]), list(fac.ap[1]), [0, D]])
                nc.vector.tensor_mul(
                    v_sb.rearrange("p (n d) -> p n d", d=D),
                    s_sb.rearrange("p (n d) -> p n d", d=D),
                    fac_b)
                return v_sb

            def allreduce_s(s_ps, r):
                """partial s (PSUM) -> allreduced SBUF fp32 tile."""
                s_loc = qp.tile([B, ND], FP, tag="s_loc")
                nc.scalar.copy(out=s_loc, in_=s_ps)
                nc.sync.dma_start(out=ar_in[r][:, :], in_=s_loc)
                nc.gpsimd.collective_compute(
                    "AllReduce",
                    mybir.AluOpType.add,
                    replica_groups=[list(range(CORES))],
                    ins=[ar_in[r][:, :].opt()],
                    outs=[ar_out[r][:, :].opt()],
                )
                s_glob = qp.tile([B, ND], FP, tag="s_glob")
                nc.sync.dma_start(out=s_glob, in_=ar_out[r][:, :])
                return s_glob

            def broadcast_v(v_sb, r):
                """v [64,1024] fp32 -> vb [128, 4*1024] bf16 (per-bg bcast)."""
                v_bf = qp.tile([B, ND], BF, tag="v_bf")
                nc.vector.tensor_copy(out=v_bf, in_=v_sb)
                nc.sync.dma_start(out=v_scr[r][:, :], in_=v_bf)
                vb = vbp.tile([128, GND], BF, tag="vb")
                for g in range(4):
                    src = bass.AP(tensor=v_scr[r], offset=g * 16 * ND,
                                  ap=[[0, 8], [ND, 16], [1, ND]])
                    nc.sync.dma_start(out=vb[:, g * ND:(g + 1) * ND], in_=src)
                return vb

            # ================= Phase P: projection + iter-0 GEMM ============
            s0_ps = psa.tile([B, ND], FP, tag="s_acc")
            for blk in range(NBLK):
                wt = wp.tile([128, WCOL], BF, tag="w")
                nc.sync.dma_start(out=wt, in_=wxbd_d[blk])
                # iter-0 s: dense GEMM over K=(s,e), accumulate over blocks
                for nh in range(2):
                    nc.tensor.matmul(
                        s0_ps[:, nh * 512:(nh + 1) * 512],
                        wt[:, ND:WCOL],
                        wt[:, nh * 512:(nh + 1) * 512],
                        start=(blk == 0), stop=(blk == NBLK - 1),
                        skip_group_check=True)
                if blk < RESB:
                    u4 = u_res[:, blk * GND:(blk + 1) * GND]
                else:
                    u4 = ulp.tile([128, GND], BF, tag="u4")
                for g in range(4):
                    u_ps = psp.tile([128, ND], FP, tag="u_ps")
                    xcol = (blk * 4 + g) * 32
                    for nh in range(2):
                        for r in range(4):
                            nc.tensor.matmul(
                                u_ps[32 * r:32 * r + 32,
                                     nh * 512:(nh + 1) * 512],
                                x2bd[32 * r:32 * r + 32, xcol:xcol + 32],
                                wt[32 * r:32 * r + 32,
                                   nh * 512:(nh + 1) * 512],
                                start=True, stop=True,
                                tile_position=(32 * r, 32 * r),
                                skip_group_check=True)
                    u_bf = u4[:, g * ND:(g + 1) * ND]
                    nc.scalar.copy(out=u_bf[:, 0:640], in_=u_ps[:, 0:640])
                    nc.vector.tensor_copy(out=u_bf[:, 640:1024],
                                          in_=u_ps[:, 640:1024])
                if blk >= RESB:
                    nc.sync.dma_start(out=u_spill_d[blk - RESB], in_=u4)

            s_glob = allreduce_s(s0_ps, 0)
            v_sb = squash(s_glob, 1.0 / NCAP)
            vb = broadcast_v(v_sb, 0)

            # ================= Routing iterations 1 and 2 ===================
            for it in (1, 2):
                s_ps = psa.tile([B, ND], FP, tag="s_acc")
                for blk in range(NBLK):
                    if blk < RESB:
                        u4 = u_res[:, blk * GND:(blk + 1) * GND]
                    else:
                        u4 = ulp.tile([128, GND], BF, tag="u4")
                        nc.sync.dma_start(out=u4, in_=u_spill_d[blk - RESB])
                    tmp4 = tp.tile([128, GND], BF, tag="tmp4")
                    nc.vector.tensor_mul(tmp4, u4, vb)
                    # in-place pairwise tree reduction over cap_dim
                    b_slice = b_all[:, blk * 4 * NCAP:(blk + 1) * 4 * NCAP]
                    tv = tmp4.rearrange("p (t d) -> p t d", d=D)
                    with nc.allow_low_precision(
                            reason="bf16 routing logits, validated offline"):
                        nc.vector.tensor_add(
                            tv[:, :, 0:8], tv[:, :, 0:8], tv[:, :, 8:16])
                        nc.vector.tensor_add(
                            tv[:, :, 0:4], tv[:, :, 0:4], tv[:, :, 4:8])
                        nc.vector.tensor_add(
                            tv[:, :, 0:2], tv[:, :, 0:2], tv[:, :, 2:4])
                        if it == 1:
                            nc.vector.tensor_add(
                                b_slice.rearrange("p (t o) -> p t o", o=1),
                                tv[:, :, 0:1], tv[:, :, 1:2])
                        else:
                            agr4 = sp.tile([128, 4 * NCAP], BF, tag="agr4")
                            nc.vector.tensor_add(
                                agr4.rearrange("p (t o) -> p t o", o=1),
                                tv[:, :, 0:1], tv[:, :, 1:2])
                            nc.vector.tensor_add(b_slice, b_slice, agr4)
                    c_un = sp.tile([128, 4 * NCAP], BF, tag="c_un")
                    nc.scalar.activation(out=c_un, in_=b_slice,
                                         func=mybir.ActivationFunctionType.Exp)
                    se4 = sp.tile([128, 4], FP, tag="se4")
                    nc.vector.reduce_sum(
                        out=se4,
                        in_=c_un.rearrange("p (t n) -> p t n", n=NCAP),
                        axis=mybir.AxisListType.X)
                    rec4 = sp.tile([128, 4], FP, tag="rec4")
                    nc.vector.reciprocal(out=rec4, in_=se4)
                    c_bf = sp.tile([128, 4 * NCAP], BF, tag="c_bf")
                    rec_b = bass.AP(tensor=rec4.tensor, offset=rec4.offset,
                                    ap=[list(rec4.ap[0]), list(rec4.ap[1]),
                                        [0, NCAP]])
                    with nc.allow_low_precision(reason="bf16 routing weights"):
                        nc.vector.tensor_mul(
                            c_bf.rearrange("p (t n) -> p t n", n=NCAP),
                            c_un.rearrange("p (t n) -> p t n", n=NCAP),
                            rec_b)
                    # replicate c over cap_dim on the scalar engine
                    c_rep = crp.tile([128, GND], BF, tag="c_rep")
                    c_src = bass.AP(tensor=c_bf.tensor, offset=c_bf.offset,
                                    ap=[list(c_bf.ap[0]), [NCAP, 4], [1, NCAP],
                                        [0, D]])
                    nc.scalar.copy(
                        out=c_rep.rearrange("p (t n d) -> p t n d",
                                            n=NCAP, d=D),
                        in_=c_src)
                    cu4 = cup.tile([128, GND], BF, tag="cu4")
                    nc.vector.tensor_mul(cu4, u4, c_rep)
                    for g in range(4):
                        t = blk * 4 + g
                        for nh in range(2):
                            nc.tensor.matmul(
                                s_ps[:, nh * 512:(nh + 1) * 512],
                                sel_sb[:, g * NCAP:(g + 1) * NCAP],
                                cu4[:, g * ND + nh * 512:g * ND + (nh + 1) * 512],
                                start=(t == 0), stop=(t == NT - 1),
                                skip_group_check=True)
                s_glob = allreduce_s(s_ps, it)
                v_sb = squash(s_glob, 1.0)
                if it < 2:
                    vb = broadcast_v(v_sb, 1)
                else:
                    nc.sync.dma_start(out=v_out[:, :], in_=v_sb)

    nc.compile()
    return nc


_CACHED = {}


def _get_program():
    if "nc" not in _CACHED:
        _CACHED["nc"] = _build_program()
    return _CACHED["nc"]


def kernel(x, W, bias):
    x = np.asarray(x, dtype=np.float32)
    W = np.asarray(W, dtype=np.float32)
    bias = np.asarray(bias, dtype=np.float32)

    wxbd_all, x2bd_all, sel, bias_flat = _host_prep(x, W, bias)
    nc = _get_program()

    in_maps = []
    for c in range(CORES):
        in_maps.append({
            "wxbd": wxbd_all[c],
            "x2bd": x2bd_all[c],
            "sel": sel,
            "bias_flat": bias_flat,
        })
    res = run_bass_kernel_spmd(nc, in_maps, core_ids=list(range(CORES)))
    _CACHED["last_results"] = res
    return res.results[0]["v_out"].reshape(B, NCAP, D).astype(np.float32)


# revision 19
# speedup vs baseline: 2.1306x; 1.0280x over previous
"""ClassCapsule dynamic-routing kernel for 8 Trainium2 NeuronCores.

Problem (hardcoded shapes):
    x:    [64, 2048, 16]  fp32
    W:    [2048, 16, 1024] fp32
    bias: [64, 16]        fp32
    out:  [64, 64, 16]    fp32  (squeezed v after 3 routing iterations)

Strategy (in_caps-sharded, s-AllReduce per iteration):
  - IC=2048 in_caps split across 8 cores (256 each); every core holds all
    64 batches.  W traffic per core is 1/8th of the replicated layout.
  - All inputs shipped bf16: per-core wxbd[blk] packs W for an 8-in_cap
    block ([128,1024]), the block-diagonal x operand ([128,512], 4 groups
    of 16 batches) and the dense x operand ([128,64]) -> one DMA feeds
    all matmul operands for a block.
  - u_hat = x @ W on the PE via the block-diagonal trick: K=(s,e)=128,
    M=(s,b16)=128 per matmul, all bf16.  Iteration-0 s (c0 uniform) is a
    plain dense GEMM over K=(in_cap,e): s0 += xk[blk].T @ W8[blk], which
    keeps the PE warm and off the copy critical path.
  - u_hat (bf16) kept SBUF-resident for the first RESB block groups,
    spilled to DRAM per block group [128,4096] and re-read in iters 1-2.
  - Per iteration, per block group: agreement products (GPSIMD for 3 of
    4 groups, DVE else), d-reduction via an in-place pairwise bf16 add
    tree on DVE (2x mode), batched softmax (ACT exp, DVE smalls), c
    replicated over cap_dim on ACT, cu product on DVE (2x), s
    accumulated with per-tile selector matmuls (PE), then an 8-core
    fp32 AllReduce of the partial s [64,1024], squash on every core.
"""

import numpy as np
import ml_dtypes

import concourse.bass as bass
import concourse.tile as tile
from concourse import bacc, mybir
from concourse.bass_utils import run_bass_kernel_spmd

# ---------------------------------------------------------------- constants
B, IC, E = 64, 2048, 16          # batch, in_caps, in_dim
NCAP, D = 64, 16                 # n_caps, cap_dim
ND = NCAP * D                    # 1024
CORES = 8
ICL = IC // CORES                # 256 local in_caps
NBLK = ICL // 8                  # 32 blocks of 8 in_caps
NT = NBLK * 4                    # 128 u_hat tiles [(s,b16), 1024]
RESB = 7                         # block groups resident in SBUF
GND = 4 * ND                     # group width (4096)
WCOL = ND + B                    # wxbd columns (1088)
EPS = 1e-7

FP = mybir.dt.float32
BF = mybir.dt.bfloat16
BF_NP = ml_dtypes.bfloat16


def _host_prep(x, W, bias):
    """Build per-core host-side tensors (all bf16 except bias)."""
    wxbd_all = []
    x2bd_all = []
    for c in range(CORES):
        i0 = c * ICL
        w8 = W[i0:i0 + ICL].reshape(NBLK, 128, ND)          # [(blk),(s,e),nd]
        xc = x[:, i0:i0 + ICL, :]                           # [64, 256, 16]
        # xr[blk, s, e, bg, b] = x[bg*16+b, i0+blk*8+s, e]
        xr = xc.transpose(1, 2, 0).reshape(NBLK, 8, E, 4, 16)
        xk = xr.reshape(NBLK, 128, B)                       # [(s,e), b]
        wxbd = np.concatenate([w8, xk], axis=2)             # [32, 128, 1088]
        wxbd_all.append(wxbd.astype(BF_NP))
        # x2bd[32r+16j'+e, (blk*4+bg)*32 + 16j + b] =
        #     x[bg*16+b, i0+blk*8+2r+j, e] * (j == j')
        xr2 = xr.reshape(NBLK, 4, 2, E, 4, 16)              # [blk,r,j,e,bg,b]
        x2 = np.zeros((4, 2, E, NBLK, 4, 2, 16), np.float32)
        for j in range(2):
            x2[:, j, :, :, :, j, :] = xr2[:, :, j].transpose(1, 2, 0, 3, 4)
        x2bd_all.append(x2.reshape(128, NBLK * 128).astype(BF_NP))

    # selector: sel[p, g*64 + m] = 1 if m == g*16 + p%16
    sel = np.zeros((128, 4 * NCAP), np.float32)
    p = np.arange(128)
    for g in range(4):
        sel[p, g * NCAP + g * 16 + (p % 16)] = 1.0
    sel = sel.astype(BF_NP)

    bias_flat = np.ascontiguousarray(bias.reshape(1, ND)).astype(np.float32)
    return wxbd_all, x2bd_all, sel, bias_flat


def _build_program():
    nc = bacc.Bacc("TRN2", target_bir_lowering=False, num_devices=CORES)

    wxbd_d = nc.dram_tensor("wxbd", [NBLK, 128, WCOL], BF,
                            kind="ExternalInput")
    x2bd_d = nc.dram_tensor("x2bd", [128, NBLK * 128], BF,
                            kind="ExternalInput")
    sel_d = nc.dram_tensor("sel", [128, 4 * NCAP], BF, kind="ExternalInput")
    bias_d = nc.dram_tensor("bias_flat", [1, ND], FP, kind="ExternalInput")
    v_out = nc.dram_tensor("v_out", [B, ND], FP, kind="ExternalOutput")

    u_spill_d = nc.dram_tensor("u_spill", [NBLK - RESB, 128, GND], BF)
    v_scr = [nc.dram_tensor(f"v_scr{r}", [B, ND], BF) for r in range(2)]
    ar_in = [nc.dram_tensor(f"ar_in{r}", [B, ND], FP) for r in range(3)]
    ar_out = [nc.dram_tensor(f"ar_out{r}", [B, ND], FP, addr_space="Shared")
              for r in range(3)]

    with tile.TileContext(nc) as tc:
        with (
            tc.tile_pool(name="wp", bufs=2) as wp,
            tc.tile_pool(name="ul", bufs=4) as ulp,        # spill load/stage
            tc.tile_pool(name="tp", bufs=2) as tp,         # u*v products
            tc.tile_pool(name="crp", bufs=2) as crp,       # c replicated
            tc.tile_pool(name="cup", bufs=2) as cup,       # u*c products
            tc.tile_pool(name="smalls", bufs=2) as sp,
            tc.tile_pool(name="sq", bufs=1) as qp,         # squash [64,*]
            tc.tile_pool(name="consts", bufs=1) as cp,
            tc.tile_pool(name="vb", bufs=1) as vbp,
            tc.tile_pool(name="ures", bufs=1) as urp,
            tc.tile_pool(name="bstate", bufs=1) as bsp,
            tc.tile_pool(name="ps", bufs=3, space="PSUM") as psp,
            tc.tile_pool(name="ps_acc", bufs=1, space="PSUM") as psa,
        ):
            # ---- constants resident in SBUF
            sel_sb = cp.tile([128, 4 * NCAP], BF)
            nc.sync.dma_start(out=sel_sb, in_=sel_d[:, :])
            x2bd = cp.tile([128, NBLK * 128], BF)
            nc.sync.dma_start(out=x2bd, in_=x2bd_d[:, :])
            bias_sb = cp.tile([B, ND], FP)
            bias_src = bass.AP(tensor=bias_d, offset=0, ap=[[0, B], [1, ND]])
            nc.sync.dma_start(out=bias_sb, in_=bias_src)
            eps_t = cp.tile([B, 1], FP)
            nc.vector.memset(eps_t, EPS)

            # persistent state
            u_res = urp.tile([128, RESB * GND], BF)        # resident u_hat
            b_all = bsp.tile([128, NT * NCAP], BF)         # routing logits

            # ---------------- squash helper: v = squash(s*scale + bias)
            def squash(s_in, scale):
                s_sb = qp.tile([B, ND], FP, tag="s_sb")
                nc.vector.scalar_tensor_tensor(
                    out=s_sb, in0=s_in, scalar=float(scale), in1=bias_sb,
                    op0=mybir.AluOpType.mult, op1=mybir.AluOpType.add)
                sq = qp.tile([B, ND], FP, tag="sq")
                nc.vector.tensor_mul(sq, s_sb, s_sb)
                nsq = sp.tile([B, NCAP], FP, tag="nsq")
                nc.vector.reduce_sum(
                    out=nsq, in_=sq.rearrange("p (n d) -> p n d", d=D),
                    axis=mybir.AxisListType.X)
                norm = sp.tile([B, NCAP], FP, tag="norm")
                nc.scalar.activation(out=norm, in_=nsq,
                                     func=mybir.ActivationFunctionType.Sqrt,
                                     bias=eps_t[:, :], scale=1.0)
                den = sp.tile([B, NCAP], FP, tag="den")
                nc.vector.scalar_tensor_tensor(
                    out=den, in0=nsq, scalar=float(EPS + 1.0), in1=norm,
                    op0=mybir.AluOpType.add, op1=mybir.AluOpType.mult)
                rden = sp.tile([B, NCAP], FP, tag="rden")
                nc.vector.reciprocal(out=rden, in_=den)
                fac = sp.tile([B, NCAP], FP, tag="fac")
                nc.vector.scalar_tensor_tensor(
                    out=fac, in0=nsq, scalar=float(EPS), in1=rden,
                    op0=mybir.AluOpType.add, op1=mybir.AluOpType.mult)
                v_sb = qp.tile([B, ND], FP, tag="v_sb")
                fac_b = bass.AP(tensor=fac.tensor, offset=fac.offset,
                                ap=[list(fac.ap[0]), list(fac.ap[1]), [0, D]])
                nc.vector.tensor_mul(
                    v_sb.rearrange("p (n d) -> p n d", d=D),
                    s_sb.rearrange("p (n d) -> p n d", d=D),
                    fac_b)
                return v_sb

            def allreduce_s(s_ps, r):
                """partial s (PSUM) -> allreduced SBUF fp32 tile."""
                s_loc = qp.tile([B, ND], FP, tag="s_loc")
                nc.scalar.copy(out=s_loc, in_=s_ps)
                nc.sync.dma_start(out=ar_in[r][:, :], in_=s_loc)
                nc.gpsimd.collective_compute(
                    "AllReduce",
                    mybir.AluOpType.add,
                    replica_groups=[list(range(CORES))],
                    ins=[ar_in[r][:, :].opt()],
                    outs=[ar_out[r][:, :].opt()],
                )
                s_glob = qp.tile([B, ND], FP, tag="s_glob")
                nc.sync.dma_start(out=s_glob, in_=ar_out[r][:, :])
                return s_glob

            def broadcast_v(v_sb, r):
                """v [64,1024] fp32 -> vb [128, 4*1024] bf16 (per-bg bcast)."""
                v_bf = qp.tile([B, ND], BF, tag="v_bf")
                nc.vector.tensor_copy(out=v_bf, in_=v_sb)
                nc.sync.dma_start(out=v_scr[r][:, :], in_=v_bf)
                vb = vbp.tile([128, GND], BF, tag="vb")
                for g in range(4):
                    src = bass.AP(tensor=v_scr[r], offset=g * 16 * ND,
                                  ap=[[0, 8], [ND, 16], [1, ND]])
                    nc.sync.dma_start(out=vb[:, g * ND:(g + 1) * ND], in_=src)
                return vb

            # ================= Phase P: projection + iter-0 GEMM ============
            s0_ps = psa.tile([B, ND], FP, tag="s_acc")
            for blk in range(NBLK):
                wt = wp.tile([128, WCOL], BF, tag="w")
                nc.sync.dma_start(out=wt, in_=wxbd_d[blk])
                # iter-0 s: dense GEMM over K=(s,e), accumulate over blocks
                for nh in range(2):
                    nc.tensor.matmul(
                        s0_ps[:, nh * 512:(nh + 1) * 512],
                        wt[:, ND:WCOL],
                        wt[:, nh * 512:(nh + 1) * 512],
                        start=(blk == 0), stop=(blk == NBLK - 1),
                        skip_group_check=True)
                if blk < RESB:
                    u4 = u_res[:, blk * GND:(blk + 1) * GND]
                else:
                    u4 = ulp.tile([128, GND], BF, tag="u4")
                for g in range(4):
                    u_ps = psp.tile([128, ND], FP, tag="u_ps")
                    xcol = (blk * 4 + g) * 32
                    for nh in range(2):
                        for r in range(4):
                            nc.tensor.matmul(
                                u_ps[32 * r:32 * r + 32,
                                     nh * 512:(nh + 1) * 512],
                                x2bd[32 * r:32 * r + 32, xcol:xcol + 32],
                                wt[32 * r:32 * r + 32,
                                   nh * 512:(nh + 1) * 512],
                                start=True, stop=True,
                                tile_position=(32 * r, 32 * r),
                                skip_group_check=True)
                    u_bf = u4[:, g * ND:(g + 1) * ND]
                    nc.scalar.copy(out=u_bf[:, 0:640], in_=u_ps[:, 0:640])
                    nc.vector.tensor_copy(out=u_bf[:, 640:1024],
                                          in_=u_ps[:, 640:1024])
                if blk >= RESB:
                    nc.sync.dma_start(out=u_spill_d[blk - RESB], in_=u4)

            s_glob = allreduce_s(s0_ps, 0)
            v_sb = squash(s_glob, 1.0 / NCAP)
            vb = broadcast_v(v_sb, 0)

            # ================= Routing iterations 1 and 2 ===================
            for it in (1, 2):
                s_ps = psa.tile([B, ND], FP, tag="s_acc")

                def stage1(blk, it=it):
                    """agreement + softmax + c-replicate for one group."""
                    if blk < RESB:
                        u4 = u_res[:, blk * GND:(blk + 1) * GND]
                    else:
                        u4 = ulp.tile([128, GND], BF, tag="u4", name="u4")
                        nc.sync.dma_start(out=u4, in_=u_spill_d[blk - RESB])
                    tmp4 = tp.tile([128, GND], BF, tag="tmp4", name="tmp4")
                    nc.vector.tensor_mul(tmp4, u4, vb)
                    # in-place pairwise tree reduction over cap_dim
                    b_slice = b_all[:, blk * 4 * NCAP:(blk + 1) * 4 * NCAP]
                    tv = tmp4.rearrange("p (t d) -> p t d", d=D)
                    with nc.allow_low_precision(
                            reason="bf16 routing logits, validated offline"):
                        nc.vector.tensor_add(
                            tv[:, :, 0:8], tv[:, :, 0:8], tv[:, :, 8:16])
                        nc.vector.tensor_add(
                            tv[:, :, 0:4], tv[:, :, 0:4], tv[:, :, 4:8])
                        nc.vector.tensor_add(
                            tv[:, :, 0:2], tv[:, :, 0:2], tv[:, :, 2:4])
                        if it == 1:
                            nc.vector.tensor_add(
                                b_slice.rearrange("p (t o) -> p t o", o=1),
                                tv[:, :, 0:1], tv[:, :, 1:2])
                        else:
                            agr4 = sp.tile([128, 4 * NCAP], BF, tag="agr4",
                                           name="agr4")
                            nc.vector.tensor_add(
                                agr4.rearrange("p (t o) -> p t o", o=1),
                                tv[:, :, 0:1], tv[:, :, 1:2])
                            nc.vector.tensor_add(b_slice, b_slice, agr4)
                    c_un = sp.tile([128, 4 * NCAP], BF, tag="c_un",
                                   name="c_un")
                    nc.scalar.activation(out=c_un, in_=b_slice,
                                         func=mybir.ActivationFunctionType.Exp)
                    se4 = sp.tile([128, 4], FP, tag="se4", name="se4")
                    nc.vector.reduce_sum(
                        out=se4,
                        in_=c_un.rearrange("p (t n) -> p t n", n=NCAP),
                        axis=mybir.AxisListType.X)
                    rec4 = sp.tile([128, 4], FP, tag="rec4", name="rec4")
                    nc.vector.reciprocal(out=rec4, in_=se4)
                    c_bf = sp.tile([128, 4 * NCAP], BF, tag="c_bf",
                                   name="c_bf")
                    rec_b = bass.AP(tensor=rec4.tensor, offset=rec4.offset,
                                    ap=[list(rec4.ap[0]), list(rec4.ap[1]),
                                        [0, NCAP]])
                    with nc.allow_low_precision(reason="bf16 routing weights"):
                        nc.vector.tensor_mul(
                            c_bf.rearrange("p (t n) -> p t n", n=NCAP),
                            c_un.rearrange("p (t n) -> p t n", n=NCAP),
                            rec_b)
                    # replicate c over cap_dim on the scalar engine
                    c_rep = crp.tile([128, GND], BF, tag="c_rep",
                                     name="c_rep")
                    c_src = bass.AP(tensor=c_bf.tensor, offset=c_bf.offset,
                                    ap=[list(c_bf.ap[0]), [NCAP, 4], [1, NCAP],
                                        [0, D]])
                    nc.scalar.copy(
                        out=c_rep.rearrange("p (t n d) -> p t n d",
                                            n=NCAP, d=D),
                        in_=c_src)
                    return u4, c_rep

                def stage2(blk, u4, c_rep):
                    """cu product + selector matmuls for one group."""
                    cu4 = cup.tile([128, GND], BF, tag="cu4", name="cu4")
                    nc.vector.tensor_mul(cu4, u4, c_rep)
                    for g in range(4):
                        t = blk * 4 + g
                        for nh in range(2):
                            nc.tensor.matmul(
                                s_ps[:, nh * 512:(nh + 1) * 512],
                                sel_sb[:, g * NCAP:(g + 1) * NCAP],
                                cu4[:, g * ND + nh * 512:g * ND + (nh + 1) * 512],
                                start=(t == 0), stop=(t == NT - 1),
                                skip_group_check=True)

                # two-stage software pipeline: the DVE's cu product for
                # group g issues after group g+1's stage-1 work, so it never
                # head-of-line blocks on the scalar engine's c-replicate.
                pending = None
                for blk in range(NBLK):
                    st = stage1(blk)
                    if pending is not None:
                        stage2(pending[0], *pending[1])
                    pending = (blk, st)
                stage2(pending[0], *pending[1])
                s_glob = allreduce_s(s_ps, it)
                v_sb = squash(s_glob, 1.0)
                if it < 2:
                    vb = broadcast_v(v_sb, 1)
                else:
                    nc.sync.dma_start(out=v_out[:, :], in_=v_sb)

    nc.compile()
    return nc


_CACHED = {}


def _get_program():
    if "nc" not in _CACHED:
        _CACHED["nc"] = _build_program()
    return _CACHED["nc"]


def kernel(x, W, bias):
    x = np.asarray(x, dtype=np.float32)
    W = np.asarray(W, dtype=np.float32)
    bias = np.asarray(bias, dtype=np.float32)

    wxbd_all, x2bd_all, sel, bias_flat = _host_prep(x, W, bias)
    nc = _get_program()

    in_maps = []
    for c in range(CORES):
        in_maps.append({
            "wxbd": wxbd_all[c],
            "x2bd": x2bd_all[c],
            "sel": sel,
            "bias_flat": bias_flat,
        })
    res = run_bass_kernel_spmd(nc, in_maps, core_ids=list(range(CORES)))
    _CACHED["last_results"] = res
    return res.results[0]["v_out"].reshape(B, NCAP, D).astype(np.float32)


# revision 21
# speedup vs baseline: 2.1346x; 1.0019x over previous
"""ClassCapsule dynamic-routing kernel for 8 Trainium2 NeuronCores.

Problem (hardcoded shapes):
    x:    [64, 2048, 16]  fp32
    W:    [2048, 16, 1024] fp32
    bias: [64, 16]        fp32
    out:  [64, 64, 16]    fp32  (squeezed v after 3 routing iterations)

Strategy (in_caps-sharded, s-AllReduce per iteration):
  - IC=2048 in_caps split across 8 cores (256 each); every core holds all
    64 batches.  W traffic per core is 1/8th of the replicated layout.
  - All inputs shipped bf16: per-core wxbd[blk] packs W for an 8-in_cap
    block ([128,1024]), the block-diagonal x operand ([128,512], 4 groups
    of 16 batches) and the dense x operand ([128,64]) -> one DMA feeds
    all matmul operands for a block.
  - u_hat = x @ W on the PE via the block-diagonal trick: K=(s,e)=128,
    M=(s,b16)=128 per matmul, all bf16.  Iteration-0 s (c0 uniform) is a
    plain dense GEMM over K=(in_cap,e): s0 += xk[blk].T @ W8[blk], which
    keeps the PE warm and off the copy critical path.
  - u_hat (bf16) kept SBUF-resident for the first RESB block groups,
    spilled to DRAM per block group [128,4096] and re-read in iters 1-2.
  - Per iteration, per block group: agreement products (GPSIMD for 3 of
    4 groups, DVE else), d-reduction via an in-place pairwise bf16 add
    tree on DVE (2x mode), batched softmax (ACT exp, DVE smalls), c
    replicated over cap_dim on ACT, cu product on DVE (2x), s
    accumulated with per-tile selector matmuls (PE), then an 8-core
    fp32 AllReduce of the partial s [64,1024], squash on every core.
"""

import numpy as np
import ml_dtypes

import concourse.bass as bass
import concourse.tile as tile
from concourse import bacc, mybir
from concourse.bass_utils import run_bass_kernel_spmd

# ---------------------------------------------------------------- constants
B, IC, E = 64, 2048, 16          # batch, in_caps, in_dim
NCAP, D = 64, 16                 # n_caps, cap_dim
ND = NCAP * D                    # 1024
CORES = 8
ICL = IC // CORES                # 256 local in_caps
NBLK = ICL // 8                  # 32 blocks of 8 in_caps
NT = NBLK * 4                    # 128 u_hat tiles [(s,b16), 1024]
RESB = 7                         # block groups resident in SBUF
GND = 4 * ND                     # group width (4096)
WCOL = ND + B                    # wxbd columns (1088)
EPS = 1e-7

FP = mybir.dt.float32
BF = mybir.dt.bfloat16
BF_NP = ml_dtypes.bfloat16


def _host_prep(x, W, bias):
    """Build per-core host-side tensors (all bf16 except bias)."""
    wxbd_all = []
    x2bd_all = []
    for c in range(CORES):
        i0 = c * ICL
        w8 = W[i0:i0 + ICL].reshape(NBLK, 128, ND)          # [(blk),(s,e),nd]
        xc = x[:, i0:i0 + ICL, :]                           # [64, 256, 16]
        # xr[blk, s, e, bg, b] = x[bg*16+b, i0+blk*8+s, e]
        xr = xc.transpose(1, 2, 0).reshape(NBLK, 8, E, 4, 16)
        xk = xr.reshape(NBLK, 128, B)                       # [(s,e), b]
        wxbd = np.concatenate([w8, xk], axis=2)             # [32, 128, 1088]
        wxbd_all.append(wxbd.astype(BF_NP))
        # x2bd[32r+16j'+e, (blk*4+bg)*32 + 16j + b] =
        #     x[bg*16+b, i0+blk*8+2r+j, e] * (j == j')
        xr2 = xr.reshape(NBLK, 4, 2, E, 4, 16)              # [blk,r,j,e,bg,b]
        x2 = np.zeros((4, 2, E, NBLK, 4, 2, 16), np.float32)
        for j in range(2):
            x2[:, j, :, :, :, j, :] = xr2[:, :, j].transpose(1, 2, 0, 3, 4)
        x2bd_all.append(x2.reshape(128, NBLK * 128).astype(BF_NP))

    # selector: sel[p, g*64 + m] = 1 if m == g*16 + p%16
    sel = np.zeros((128, 4 * NCAP), np.float32)
    p = np.arange(128)
    for g in range(4):
        sel[p, g * NCAP + g * 16 + (p % 16)] = 1.0
    sel = sel.astype(BF_NP)

    bias_flat = np.ascontiguousarray(bias.reshape(1, ND)).astype(np.float32)
    return wxbd_all, x2bd_all, sel, bias_flat


def _build_program():
    nc = bacc.Bacc("TRN2", target_bir_lowering=False, num_devices=CORES)

    wxbd_d = nc.dram_tensor("wxbd", [NBLK, 128, WCOL], BF,
                            kind="ExternalInput")
    x2bd_d = nc.dram_tensor("x2bd", [128, NBLK * 128], BF,
                            kind="ExternalInput")
    sel_d = nc.dram_tensor("sel", [128, 4 * NCAP], BF, kind="ExternalInput")
    bias_d = nc.dram_tensor("bias_flat", [1, ND], FP, kind="ExternalInput")
    v_out = nc.dram_tensor("v_out", [B, ND], FP, kind="ExternalOutput")

    u_spill_d = nc.dram_tensor("u_spill", [NBLK - RESB, 128, GND], BF)
    v_scr = [nc.dram_tensor(f"v_scr{r}", [B, ND], BF) for r in range(2)]
    ar_in = [nc.dram_tensor(f"ar_in{r}", [B, ND], FP) for r in range(3)]
    ar_out = [nc.dram_tensor(f"ar_out{r}", [B, ND], FP, addr_space="Shared")
              for r in range(3)]

    with tile.TileContext(nc) as tc:
        with (
            tc.tile_pool(name="wp", bufs=2) as wp,
            tc.tile_pool(name="ul", bufs=4) as ulp,        # spill load/stage
            tc.tile_pool(name="tp", bufs=2) as tp,         # u*v products
            tc.tile_pool(name="crp", bufs=2) as crp,       # c replicated
            tc.tile_pool(name="cup", bufs=2) as cup,       # u*c products
            tc.tile_pool(name="smalls", bufs=2) as sp,
            tc.tile_pool(name="sq", bufs=1) as qp,         # squash [64,*]
            tc.tile_pool(name="consts", bufs=1) as cp,
            tc.tile_pool(name="vb", bufs=1) as vbp,
            tc.tile_pool(name="ures", bufs=1) as urp,
            tc.tile_pool(name="bstate", bufs=1) as bsp,
            tc.tile_pool(name="ps", bufs=3, space="PSUM") as psp,
            tc.tile_pool(name="ps_acc", bufs=1, space="PSUM") as psa,
        ):
            # ---- constants resident in SBUF
            sel_sb = cp.tile([128, 4 * NCAP], BF)
            nc.sync.dma_start(out=sel_sb, in_=sel_d[:, :])
            x2bd = cp.tile([128, NBLK * 128], BF)
            nc.sync.dma_start(out=x2bd, in_=x2bd_d[:, :])
            bias_sb = cp.tile([B, ND], FP)
            bias_src = bass.AP(tensor=bias_d, offset=0, ap=[[0, B], [1, ND]])
            nc.sync.dma_start(out=bias_sb, in_=bias_src)
            eps_t = cp.tile([B, 1], FP)
            nc.vector.memset(eps_t, EPS)

            # persistent state
            u_res = urp.tile([128, RESB * GND], BF)        # resident u_hat
            b_all = bsp.tile([128, NT * NCAP], BF)         # routing logits

            # ---------------- squash helper: v = squash(s*scale + bias)
            def squash(s_in, scale, bf16=False):
                s_sb = qp.tile([B, ND], FP, tag="s_sb")
                nc.vector.scalar_tensor_tensor(
                    out=s_sb, in0=s_in, scalar=float(scale), in1=bias_sb,
                    op0=mybir.AluOpType.mult, op1=mybir.AluOpType.add)
                sq = qp.tile([B, ND], FP, tag="sq")
                nc.vector.tensor_mul(sq, s_sb, s_sb)
                nsq = sp.tile([B, NCAP], FP, tag="nsq")
                nc.vector.reduce_sum(
                    out=nsq, in_=sq.rearrange("p (n d) -> p n d", d=D),
                    axis=mybir.AxisListType.X)
                norm = sp.tile([B, NCAP], FP, tag="norm")
                nc.scalar.activation(out=norm, in_=nsq,
                                     func=mybir.ActivationFunctionType.Sqrt,
                                     bias=eps_t[:, :], scale=1.0)
                den = sp.tile([B, NCAP], FP, tag="den")
                nc.vector.scalar_tensor_tensor(
                    out=den, in0=nsq, scalar=float(EPS + 1.0), in1=norm,
                    op0=mybir.AluOpType.add, op1=mybir.AluOpType.mult)
                rden = sp.tile([B, NCAP], FP, tag="rden")
                nc.vector.reciprocal(out=rden, in_=den)
                fac = sp.tile([B, NCAP], FP, tag="fac")
                nc.vector.scalar_tensor_tensor(
                    out=fac, in0=nsq, scalar=float(EPS), in1=rden,
                    op0=mybir.AluOpType.add, op1=mybir.AluOpType.mult)
                if bf16:
                    out_bf16 = qp.tile([B, ND], BF, tag="v_bf16",
                                       name="v_bf16")
                else:
                    out_bf16 = qp.tile([B, ND], FP, tag="v_sb", name="v_sb")
                fac_b = bass.AP(tensor=fac.tensor, offset=fac.offset,
                                ap=[list(fac.ap[0]), list(fac.ap[1]), [0, D]])
                with nc.allow_low_precision(reason="bf16 v for broadcast"):
                    nc.vector.tensor_mul(
                        out_bf16.rearrange("p (n d) -> p n d", d=D),
                        s_sb.rearrange("p (n d) -> p n d", d=D),
                        fac_b)
                return out_bf16

            def allreduce_s(s_ps, r):
                """partial s (PSUM) -> allreduced SBUF fp32 tile."""
                s_loc = qp.tile([B, ND], FP, tag="s_loc")
                nc.scalar.copy(out=s_loc, in_=s_ps)
                nc.sync.dma_start(out=ar_in[r][:, :], in_=s_loc)
                nc.gpsimd.collective_compute(
                    "AllReduce",
                    mybir.AluOpType.add,
                    replica_groups=[list(range(CORES))],
                    ins=[ar_in[r][:, :].opt()],
                    outs=[ar_out[r][:, :].opt()],
                )
                s_glob = qp.tile([B, ND], FP, tag="s_glob")
                nc.sync.dma_start(out=s_glob, in_=ar_out[r][:, :])
                return s_glob

            def broadcast_v(v_bf, r):
                """v [64,1024] bf16 -> vb [128, 4*1024] bf16 (per-bg bcast)."""
                nc.sync.dma_start(out=v_scr[r][:, :], in_=v_bf)
                vb = vbp.tile([128, GND], BF, tag="vb")
                for g in range(4):
                    src = bass.AP(tensor=v_scr[r], offset=g * 16 * ND,
                                  ap=[[0, 8], [ND, 16], [1, ND]])
                    nc.sync.dma_start(out=vb[:, g * ND:(g + 1) * ND], in_=src)
                return vb

            # ================= Phase P: projection + iter-0 GEMM ============
            s0_ps = psa.tile([B, ND], FP, tag="s_acc")
            for blk in range(NBLK):
                wt = wp.tile([128, WCOL], BF, tag="w")
                nc.sync.dma_start(out=wt, in_=wxbd_d[blk])
                # iter-0 s: dense GEMM over K=(s,e), accumulate over blocks
                for nh in range(2):
                    nc.tensor.matmul(
                        s0_ps[:, nh * 512:(nh + 1) * 512],
                        wt[:, ND:WCOL],
                        wt[:, nh * 512:(nh + 1) * 512],
                        start=(blk == 0), stop=(blk == NBLK - 1),
                        skip_group_check=True)
                if blk < RESB:
                    u4 = u_res[:, blk * GND:(blk + 1) * GND]
                else:
                    u4 = ulp.tile([128, GND], BF, tag="u4")
                for g in range(4):
                    u_ps = psp.tile([128, ND], FP, tag="u_ps")
                    xcol = (blk * 4 + g) * 32
                    for nh in range(2):
                        for r in range(4):
                            nc.tensor.matmul(
                                u_ps[32 * r:32 * r + 32,
                                     nh * 512:(nh + 1) * 512],
                                x2bd[32 * r:32 * r + 32, xcol:xcol + 32],
                                wt[32 * r:32 * r + 32,
                                   nh * 512:(nh + 1) * 512],
                                start=True, stop=True,
                                tile_position=(32 * r, 32 * r),
                                skip_group_check=True)
                    u_bf = u4[:, g * ND:(g + 1) * ND]
                    nc.scalar.copy(out=u_bf[:, 0:576], in_=u_ps[:, 0:576])
                    nc.vector.tensor_copy(out=u_bf[:, 576:1024],
                                          in_=u_ps[:, 576:1024])
                if blk >= RESB:
                    nc.sync.dma_start(out=u_spill_d[blk - RESB], in_=u4)

            s_glob = allreduce_s(s0_ps, 0)
            v_sb = squash(s_glob, 1.0 / NCAP, bf16=True)
            vb = broadcast_v(v_sb, 0)

            # ================= Routing iterations 1 and 2 ===================
            for it in (1, 2):
                s_ps = psa.tile([B, ND], FP, tag="s_acc")

                def stage1(blk, it=it):
                    """agreement + softmax + c-replicate for one group."""
                    if blk < RESB:
                        u4 = u_res[:, blk * GND:(blk + 1) * GND]
                    else:
                        u4 = ulp.tile([128, GND], BF, tag="u4", name="u4")
                        nc.sync.dma_start(out=u4, in_=u_spill_d[blk - RESB])
                    tmp4 = tp.tile([128, GND], BF, tag="tmp4", name="tmp4")
                    nc.vector.tensor_mul(tmp4, u4, vb)
                    # in-place pairwise tree reduction over cap_dim
                    b_slice = b_all[:, blk * 4 * NCAP:(blk + 1) * 4 * NCAP]
                    tv = tmp4.rearrange("p (t d) -> p t d", d=D)
                    with nc.allow_low_precision(
                            reason="bf16 routing logits, validated offline"):
                        nc.vector.tensor_add(
                            tv[:, :, 0:8], tv[:, :, 0:8], tv[:, :, 8:16])
                        nc.vector.tensor_add(
                            tv[:, :, 0:4], tv[:, :, 0:4], tv[:, :, 4:8])
                        nc.vector.tensor_add(
                            tv[:, :, 0:2], tv[:, :, 0:2], tv[:, :, 2:4])
                        if it == 1:
                            nc.vector.tensor_add(
                                b_slice.rearrange("p (t o) -> p t o", o=1),
                                tv[:, :, 0:1], tv[:, :, 1:2])
                        else:
                            agr4 = sp.tile([128, 4 * NCAP], BF, tag="agr4",
                                           name="agr4")
                            nc.vector.tensor_add(
                                agr4.rearrange("p (t o) -> p t o", o=1),
                                tv[:, :, 0:1], tv[:, :, 1:2])
                            nc.vector.tensor_add(b_slice, b_slice, agr4)
                    c_un = sp.tile([128, 4 * NCAP], BF, tag="c_un",
                                   name="c_un")
                    nc.scalar.activation(out=c_un, in_=b_slice,
                                         func=mybir.ActivationFunctionType.Exp)
                    se4 = sp.tile([128, 4], FP, tag="se4", name="se4")
                    nc.vector.reduce_sum(
                        out=se4,
                        in_=c_un.rearrange("p (t n) -> p t n", n=NCAP),
                        axis=mybir.AxisListType.X)
                    rec4 = sp.tile([128, 4], FP, tag="rec4", name="rec4")
                    nc.vector.reciprocal(out=rec4, in_=se4)
                    c_bf = sp.tile([128, 4 * NCAP], BF, tag="c_bf",
                                   name="c_bf")
                    rec_b = bass.AP(tensor=rec4.tensor, offset=rec4.offset,
                                    ap=[list(rec4.ap[0]), list(rec4.ap[1]),
                                        [0, NCAP]])
                    with nc.allow_low_precision(reason="bf16 routing weights"):
                        nc.vector.tensor_mul(
                            c_bf.rearrange("p (t n) -> p t n", n=NCAP),
                            c_un.rearrange("p (t n) -> p t n", n=NCAP),
                            rec_b)
                    # replicate c over cap_dim on the scalar engine
                    c_rep = crp.tile([128, GND], BF, tag="c_rep",
                                     name="c_rep")
                    c_src = bass.AP(tensor=c_bf.tensor, offset=c_bf.offset,
                                    ap=[list(c_bf.ap[0]), [NCAP, 4], [1, NCAP],
                                        [0, D]])
                    nc.scalar.copy(
                        out=c_rep.rearrange("p (t n d) -> p t n d",
                                            n=NCAP, d=D),
                        in_=c_src)
                    return u4, c_rep

                def stage2(blk, u4, c_rep):
                    """cu product + selector matmuls for one group."""
                    cu4 = cup.tile([128, GND], BF, tag="cu4", name="cu4")
                    nc.vector.tensor_mul(cu4, u4, c_rep)
                    for g in range(4):
                        t = blk * 4 + g
                        for nh in range(2):
                            nc.tensor.matmul(
                                s_ps[:, nh * 512:(nh + 1) * 512],
                                sel_sb[:, g * NCAP:(g + 1) * NCAP],
                                cu4[:, g * ND + nh * 512:g * ND + (nh + 1) * 512],
                                start=(t == 0), stop=(t == NT - 1),
                                skip_group_check=True)

                # two-stage software pipeline: the DVE's cu product for
                # group g issues after group g+1's stage-1 work, so it never
                # head-of-line blocks on the scalar engine's c-replicate.
                pending = None
                for blk in range(NBLK):
                    st = stage1(blk)
                    if pending is not None:
                        stage2(pending[0], *pending[1])
                    pending = (blk, st)
                stage2(pending[0], *pending[1])
                s_glob = allreduce_s(s_ps, it)
                v_sb = squash(s_glob, 1.0, bf16=(it < 2))
                if it < 2:
                    vb = broadcast_v(v_sb, 1)
                else:
                    nc.sync.dma_start(out=v_out[:, :], in_=v_sb)

    nc.compile()
    return nc


_CACHED = {}


def _get_program():
    if "nc" not in _CACHED:
        _CACHED["nc"] = _build_program()
    return _CACHED["nc"]


def kernel(x, W, bias):
    x = np.asarray(x, dtype=np.float32)
    W = np.asarray(W, dtype=np.float32)
    bias = np.asarray(bias, dtype=np.float32)

    wxbd_all, x2bd_all, sel, bias_flat = _host_prep(x, W, bias)
    nc = _get_program()

    in_maps = []
    for c in range(CORES):
        in_maps.append({
            "wxbd": wxbd_all[c],
            "x2bd": x2bd_all[c],
            "sel": sel,
            "bias_flat": bias_flat,
        })
    res = run_bass_kernel_spmd(nc, in_maps, core_ids=list(range(CORES)))
    _CACHED["last_results"] = res
    return res.results[0]["v_out"].reshape(B, NCAP, D).astype(np.float32)


# revision 22
# speedup vs baseline: 2.1645x; 1.0140x over previous
"""ClassCapsule dynamic-routing kernel for 8 Trainium2 NeuronCores.

Problem (hardcoded shapes):
    x:    [64, 2048, 16]  fp32
    W:    [2048, 16, 1024] fp32
    bias: [64, 16]        fp32
    out:  [64, 64, 16]    fp32  (squeezed v after 3 routing iterations)

Strategy (in_caps-sharded, s-AllReduce per iteration):
  - IC=2048 in_caps split across 8 cores (256 each); every core holds all
    64 batches.  W traffic per core is 1/8th of the replicated layout.
  - All inputs shipped bf16: per-core wxbd[blk] packs W for an 8-in_cap
    block ([128,1024]), the block-diagonal x operand ([128,512], 4 groups
    of 16 batches) and the dense x operand ([128,64]) -> one DMA feeds
    all matmul operands for a block.
  - u_hat = x @ W on the PE via the block-diagonal trick: K=(s,e)=128,
    M=(s,b16)=128 per matmul, all bf16.  Iteration-0 s (c0 uniform) is a
    plain dense GEMM over K=(in_cap,e): s0 += xk[blk].T @ W8[blk], which
    keeps the PE warm and off the copy critical path.
  - u_hat (bf16) kept SBUF-resident for the first RESB block groups,
    spilled to DRAM per block group [128,4096] and re-read in iters 1-2.
  - Per iteration, per block group: agreement products (GPSIMD for 3 of
    4 groups, DVE else), d-reduction via an in-place pairwise bf16 add
    tree on DVE (2x mode), batched softmax (ACT exp, DVE smalls), c
    replicated over cap_dim on ACT, cu product on DVE (2x), s
    accumulated with per-tile selector matmuls (PE), then an 8-core
    fp32 AllReduce of the partial s [64,1024], squash on every core.
"""

import numpy as np
import ml_dtypes

import concourse.bass as bass
import concourse.tile as tile
from concourse import bacc, mybir
from concourse.bass_utils import run_bass_kernel_spmd

# ---------------------------------------------------------------- constants
B, IC, E = 64, 2048, 16          # batch, in_caps, in_dim
NCAP, D = 64, 16                 # n_caps, cap_dim
ND = NCAP * D                    # 1024
CORES = 8
ICL = IC // CORES                # 256 local in_caps
NBLK = ICL // 8                  # 32 blocks of 8 in_caps
NT = NBLK * 4                    # 128 u_hat tiles [(s,b16), 1024]
RESB = 7                         # block groups resident in SBUF
GND = 4 * ND                     # group width (4096)
WCOL = ND + B                    # wxbd columns (1088)
EPS = 1e-7

FP = mybir.dt.float32
BF = mybir.dt.bfloat16
BF_NP = ml_dtypes.bfloat16


def _host_prep(x, W, bias):
    """Build per-core host-side tensors (all bf16 except bias)."""
    wxbd_all = []
    x2bd_all = []
    for c in range(CORES):
        i0 = c * ICL
        w8 = W[i0:i0 + ICL].reshape(NBLK, 128, ND)          # [(blk),(s,e),nd]
        xc = x[:, i0:i0 + ICL, :]                           # [64, 256, 16]
        # xr[blk, s, e, bg, b] = x[bg*16+b, i0+blk*8+s, e]
        xr = xc.transpose(1, 2, 0).reshape(NBLK, 8, E, 4, 16)
        xk = xr.reshape(NBLK, 128, B)                       # [(s,e), b]
        wxbd = np.concatenate([w8, xk], axis=2)             # [32, 128, 1088]
        wxbd_all.append(wxbd.astype(BF_NP))
        # x2bd[32r+16j'+e, (blk*4+bg)*32 + 16j + b] =
        #     x[bg*16+b, i0+blk*8+2r+j, e] * (j == j')
        xr2 = xr.reshape(NBLK, 4, 2, E, 4, 16)              # [blk,r,j,e,bg,b]
        x2 = np.zeros((4, 2, E, NBLK, 4, 2, 16), np.float32)
        for j in range(2):
            x2[:, j, :, :, :, j, :] = xr2[:, :, j].transpose(1, 2, 0, 3, 4)
        x2bd_all.append(x2.reshape(128, NBLK * 128).astype(BF_NP))

    # selector: sel[p, g*64 + m] = 1 if m == g*16 + p%16
    sel = np.zeros((128, 4 * NCAP), np.float32)
    p = np.arange(128)
    for g in range(4):
        sel[p, g * NCAP + g * 16 + (p % 16)] = 1.0
    sel = sel.astype(BF_NP)

    bias_flat = np.ascontiguousarray(bias.reshape(1, ND)).astype(np.float32)
    return wxbd_all, x2bd_all, sel, bias_flat


def _build_program():
    nc = bacc.Bacc("TRN2", target_bir_lowering=False, num_devices=CORES)

    wxbd_d = nc.dram_tensor("wxbd", [NBLK, 128, WCOL], BF,
                            kind="ExternalInput")
    x2bd_d = nc.dram_tensor("x2bd", [128, NBLK * 128], BF,
                            kind="ExternalInput")
    sel_d = nc.dram_tensor("sel", [128, 4 * NCAP], BF, kind="ExternalInput")
    bias_d = nc.dram_tensor("bias_flat", [1, ND], FP, kind="ExternalInput")
    v_out = nc.dram_tensor("v_out", [B, ND], FP, kind="ExternalOutput")

    u_spill_d = nc.dram_tensor("u_spill", [NBLK - RESB, 128, GND], BF)
    v_scr = [nc.dram_tensor(f"v_scr{r}", [B, ND], BF) for r in range(2)]
    ar_in = [nc.dram_tensor(f"ar_in{r}", [B, ND], FP) for r in range(3)]
    ar_out = [nc.dram_tensor(f"ar_out{r}", [B, ND], FP, addr_space="Shared")
              for r in range(3)]

    with tile.TileContext(nc) as tc:
        with (
            tc.tile_pool(name="wp", bufs=2) as wp,
            tc.tile_pool(name="ul", bufs=4) as ulp,        # spill load/stage
            tc.tile_pool(name="tp", bufs=2) as tp,         # u*v products
            tc.tile_pool(name="crp", bufs=2) as crp,       # c replicated
            tc.tile_pool(name="cup", bufs=2) as cup,       # u*c products
            tc.tile_pool(name="smalls", bufs=2) as sp,
            tc.tile_pool(name="sq", bufs=1) as qp,         # squash [64,*]
            tc.tile_pool(name="consts", bufs=1) as cp,
            tc.tile_pool(name="vb", bufs=1) as vbp,
            tc.tile_pool(name="ures", bufs=1) as urp,
            tc.tile_pool(name="bstate", bufs=1) as bsp,
            tc.tile_pool(name="ps", bufs=3, space="PSUM") as psp,
            tc.tile_pool(name="ps_acc", bufs=1, space="PSUM") as psa,
        ):
            # ---- constants resident in SBUF
            sel_sb = cp.tile([128, 4 * NCAP], BF)
            nc.sync.dma_start(out=sel_sb, in_=sel_d[:, :])
            x2bd = cp.tile([128, NBLK * 128], BF)
            nc.sync.dma_start(out=x2bd, in_=x2bd_d[:, :])
            bias_sb = cp.tile([B, ND], FP)
            bias_src = bass.AP(tensor=bias_d, offset=0, ap=[[0, B], [1, ND]])
            nc.sync.dma_start(out=bias_sb, in_=bias_src)
            eps_t = cp.tile([B, 1], FP)
            nc.vector.memset(eps_t, EPS)

            # persistent state
            u_res = urp.tile([128, RESB * GND], BF)        # resident u_hat
            b_all = bsp.tile([128, NT * NCAP], BF)         # routing logits

            # ---------------- squash helper: v = squash(s*scale + bias)
            def squash(s_in, scale, bf16=False):
                s_sb = qp.tile([B, ND], FP, tag="s_sb")
                nc.vector.scalar_tensor_tensor(
                    out=s_sb, in0=s_in, scalar=float(scale), in1=bias_sb,
                    op0=mybir.AluOpType.mult, op1=mybir.AluOpType.add)
                sq = qp.tile([B, ND], FP, tag="sq")
                nc.vector.tensor_mul(sq, s_sb, s_sb)
                nsq = sp.tile([B, NCAP], FP, tag="nsq")
                nc.vector.reduce_sum(
                    out=nsq, in_=sq.rearrange("p (n d) -> p n d", d=D),
                    axis=mybir.AxisListType.X)
                norm = sp.tile([B, NCAP], FP, tag="norm")
                nc.scalar.activation(out=norm, in_=nsq,
                                     func=mybir.ActivationFunctionType.Sqrt,
                                     bias=eps_t[:, :], scale=1.0)
                den = sp.tile([B, NCAP], FP, tag="den")
                nc.vector.scalar_tensor_tensor(
                    out=den, in0=nsq, scalar=float(EPS + 1.0), in1=norm,
                    op0=mybir.AluOpType.add, op1=mybir.AluOpType.mult)
                rden = sp.tile([B, NCAP], FP, tag="rden")
                nc.vector.reciprocal(out=rden, in_=den)
                fac = sp.tile([B, NCAP], FP, tag="fac")
                nc.vector.scalar_tensor_tensor(
                    out=fac, in0=nsq, scalar=float(EPS), in1=rden,
                    op0=mybir.AluOpType.add, op1=mybir.AluOpType.mult)
                if bf16:
                    out_bf16 = qp.tile([B, ND], BF, tag="v_bf16",
                                       name="v_bf16")
                else:
                    out_bf16 = qp.tile([B, ND], FP, tag="v_sb", name="v_sb")
                fac_b = bass.AP(tensor=fac.tensor, offset=fac.offset,
                                ap=[list(fac.ap[0]), list(fac.ap[1]), [0, D]])
                with nc.allow_low_precision(reason="bf16 v for broadcast"):
                    nc.vector.tensor_mul(
                        out_bf16.rearrange("p (n d) -> p n d", d=D),
                        s_sb.rearrange("p (n d) -> p n d", d=D),
                        fac_b)
                return out_bf16

            def allreduce_s(s_ps, r):
                """partial s (PSUM) -> allreduced SBUF fp32 tile."""
                s_loc = qp.tile([B, ND], FP, tag="s_loc")
                nc.scalar.copy(out=s_loc, in_=s_ps)
                nc.sync.dma_start(out=ar_in[r][:, :], in_=s_loc)
                nc.gpsimd.collective_compute(
                    "AllReduce",
                    mybir.AluOpType.add,
                    replica_groups=[list(range(CORES))],
                    ins=[ar_in[r][:, :].opt()],
                    outs=[ar_out[r][:, :].opt()],
                )
                s_glob = qp.tile([B, ND], FP, tag="s_glob")
                nc.scalar.dma_start(out=s_glob, in_=ar_out[r][:, :])
                return s_glob

            def broadcast_v(v_bf, r):
                """v [64,1024] bf16 -> vb [128, 4*1024] bf16 (per-bg bcast)."""
                nc.sync.dma_start(out=v_scr[r][:, :], in_=v_bf)
                vb = vbp.tile([128, GND], BF, tag="vb")
                for g in range(4):
                    src = bass.AP(tensor=v_scr[r], offset=g * 16 * ND,
                                  ap=[[0, 8], [ND, 16], [1, ND]])
                    eng = nc.sync if g % 2 == 0 else nc.scalar
                    eng.dma_start(out=vb[:, g * ND:(g + 1) * ND], in_=src)
                return vb

            # ================= Phase P: projection + iter-0 GEMM ============
            s0_ps = psa.tile([B, ND], FP, tag="s_acc")
            for blk in range(NBLK):
                wt = wp.tile([128, WCOL], BF, tag="w")
                nc.sync.dma_start(out=wt, in_=wxbd_d[blk])
                # iter-0 s: dense GEMM over K=(s,e), accumulate over blocks
                for nh in range(2):
                    nc.tensor.matmul(
                        s0_ps[:, nh * 512:(nh + 1) * 512],
                        wt[:, ND:WCOL],
                        wt[:, nh * 512:(nh + 1) * 512],
                        start=(blk == 0), stop=(blk == NBLK - 1),
                        skip_group_check=True)
                if blk < RESB:
                    u4 = u_res[:, blk * GND:(blk + 1) * GND]
                else:
                    u4 = ulp.tile([128, GND], BF, tag="u4")
                for g in range(4):
                    u_ps = psp.tile([128, ND], FP, tag="u_ps")
                    xcol = (blk * 4 + g) * 32
                    for nh in range(2):
                        for r in range(4):
                            nc.tensor.matmul(
                                u_ps[32 * r:32 * r + 32,
                                     nh * 512:(nh + 1) * 512],
                                x2bd[32 * r:32 * r + 32, xcol:xcol + 32],
                                wt[32 * r:32 * r + 32,
                                   nh * 512:(nh + 1) * 512],
                                start=True, stop=True,
                                tile_position=(32 * r, 32 * r),
                                skip_group_check=True)
                    u_bf = u4[:, g * ND:(g + 1) * ND]
                    nc.scalar.copy(out=u_bf[:, 0:576], in_=u_ps[:, 0:576])
                    nc.vector.tensor_copy(out=u_bf[:, 576:1024],
                                          in_=u_ps[:, 576:1024])
                if blk >= RESB:
                    nc.sync.dma_start(out=u_spill_d[blk - RESB], in_=u4)

            s_glob = allreduce_s(s0_ps, 0)
            v_sb = squash(s_glob, 1.0 / NCAP, bf16=True)
            vb = broadcast_v(v_sb, 0)

            # ================= Routing iterations 1 and 2 ===================
            for it in (1, 2):
                s_ps = psa.tile([B, ND], FP, tag="s_acc")

                def stage1(blk, it=it):
                    """agreement + softmax + c-replicate for one group."""
                    if blk < RESB:
                        u4 = u_res[:, blk * GND:(blk + 1) * GND]
                    else:
                        u4 = ulp.tile([128, GND], BF, tag="u4", name="u4")
                        nc.sync.dma_start(out=u4, in_=u_spill_d[blk - RESB])
                    tmp4 = tp.tile([128, GND], BF, tag="tmp4", name="tmp4")
                    nc.vector.tensor_mul(tmp4, u4, vb)
                    # in-place pairwise tree reduction over cap_dim
                    b_slice = b_all[:, blk * 4 * NCAP:(blk + 1) * 4 * NCAP]
                    tv = tmp4.rearrange("p (t d) -> p t d", d=D)
                    with nc.allow_low_precision(
                            reason="bf16 routing logits, validated offline"):
                        nc.vector.tensor_add(
                            tv[:, :, 0:8], tv[:, :, 0:8], tv[:, :, 8:16])
                        nc.vector.tensor_add(
                            tv[:, :, 0:4], tv[:, :, 0:4], tv[:, :, 4:8])
                        nc.vector.tensor_add(
                            tv[:, :, 0:2], tv[:, :, 0:2], tv[:, :, 2:4])
                        if it == 1:
                            nc.vector.tensor_add(
                                b_slice.rearrange("p (t o) -> p t o", o=1),
                                tv[:, :, 0:1], tv[:, :, 1:2])
                        else:
                            agr4 = sp.tile([128, 4 * NCAP], BF, tag="agr4",
                                           name="agr4")
                            nc.vector.tensor_add(
                                agr4.rearrange("p (t o) -> p t o", o=1),
                                tv[:, :, 0:1], tv[:, :, 1:2])
                            nc.vector.tensor_add(b_slice, b_slice, agr4)
                    c_un = sp.tile([128, 4 * NCAP], BF, tag="c_un",
                                   name="c_un")
                    nc.scalar.activation(out=c_un, in_=b_slice,
                                         func=mybir.ActivationFunctionType.Exp)
                    se4 = sp.tile([128, 4], FP, tag="se4", name="se4")
                    nc.vector.reduce_sum(
                        out=se4,
                        in_=c_un.rearrange("p (t n) -> p t n", n=NCAP),
                        axis=mybir.AxisListType.X)
                    rec4 = sp.tile([128, 4], FP, tag="rec4", name="rec4")
                    nc.vector.reciprocal(out=rec4, in_=se4)
                    c_bf = sp.tile([128, 4 * NCAP], BF, tag="c_bf",
                                   name="c_bf")
                    rec_b = bass.AP(tensor=rec4.tensor, offset=rec4.offset,
                                    ap=[list(rec4.ap[0]), list(rec4.ap[1]),
                                        [0, NCAP]])
                    with nc.allow_low_precision(reason="bf16 routing weights"):
                        nc.vector.tensor_mul(
                            c_bf.rearrange("p (t n) -> p t n", n=NCAP),
                            c_un.rearrange("p (t n) -> p t n", n=NCAP),
                            rec_b)
                    # replicate c over cap_dim on the scalar engine
                    c_rep = crp.tile([128, GND], BF, tag="c_rep",
                                     name="c_rep")
                    c_src = bass.AP(tensor=c_bf.tensor, offset=c_bf.offset,
                                    ap=[list(c_bf.ap[0]), [NCAP, 4], [1, NCAP],
                                        [0, D]])
                    nc.scalar.copy(
                        out=c_rep.rearrange("p (t n d) -> p t n d",
                                            n=NCAP, d=D),
                        in_=c_src)
                    return u4, c_rep

                def stage2(blk, u4, c_rep):
                    """cu product + selector matmuls for one group."""
                    cu4 = cup.tile([128, GND], BF, tag="cu4", name="cu4")
                    nc.vector.tensor_mul(cu4, u4, c_rep)
                    for g in range(4):
                        t = blk * 4 + g
                        for nh in range(2):
                            nc.tensor.matmul(
                                s_ps[:, nh * 512:(nh + 1) * 512],
                                sel_sb[:, g * NCAP:(g + 1) * NCAP],
                                cu4[:, g * ND + nh * 512:g * ND + (nh + 1) * 512],
                                start=(t == 0), stop=(t == NT - 1),
                                skip_group_check=True)

                # two-stage software pipeline: the DVE's cu product for
                # group g issues after group g+1's stage-1 work, so it never
                # head-of-line blocks on the scalar engine's c-replicate.
                pending = None
                for blk in range(NBLK):
                    st = stage1(blk)
                    if pending is not None:
                        stage2(pending[0], *pending[1])
                    pending = (blk, st)
                stage2(pending[0], *pending[1])
                s_glob = allreduce_s(s_ps, it)
                v_sb = squash(s_glob, 1.0, bf16=(it < 2))
                if it < 2:
                    vb = broadcast_v(v_sb, 1)
                else:
                    nc.sync.dma_start(out=v_out[:, :], in_=v_sb)

    nc.compile()
    return nc


_CACHED = {}


def _get_program():
    if "nc" not in _CACHED:
        _CACHED["nc"] = _build_program()
    return _CACHED["nc"]


def kernel(x, W, bias):
    x = np.asarray(x, dtype=np.float32)
    W = np.asarray(W, dtype=np.float32)
    bias = np.asarray(bias, dtype=np.float32)

    wxbd_all, x2bd_all, sel, bias_flat = _host_prep(x, W, bias)
    nc = _get_program()

    in_maps = []
    for c in range(CORES):
        in_maps.append({
            "wxbd": wxbd_all[c],
            "x2bd": x2bd_all[c],
            "sel": sel,
            "bias_flat": bias_flat,
        })
    res = run_bass_kernel_spmd(nc, in_maps, core_ids=list(range(CORES)))
    _CACHED["last_results"] = res
    return res.results[0]["v_out"].reshape(B, NCAP, D).astype(np.float32)
